# revision 23
# baseline (speedup 1.0000x reference)
"""Self-contained Trainium2 kernel for nn_Block (dense transformer block),
8-way batch-parallel across NeuronCores.  V2: fp8e4m3 DoubleRow matmuls
for qkv/qkT/proj/fc1/fc2, token-major fc2 (no output transposes),
softmax denominators via DVE reciprocal (off the ACT critical path),
exp in [128,1024] tiles.

Per-core program: one transformer block over one batch element
x[1024, 768] -> out[1024, 768].

Layouts: token-major = tokens on partitions; feature-major = channels on
partitions.  LN runs token-major (free-dim stats via bn_stats), then
PE-transposes into feature-major with the LN gain/bias fused into the
PSUM evacuation (per-partition scale/bias APs), quantizing to fp8 in
"pair" layout [128, 2, N] where slot i holds channel rows 256j+128i+p
(DoubleRow contraction over K=256 per matmul pass).

Weights arrive in DRAM pre-cast to fp8e4m3 in matching pair layout,
pre-scaled x16 (x32 for fc2) so U(-1/sqrt(fan_in),..) values clear
e4m3's subnormal cutoff; descales fold into exp's scale, gelu's input
scale, and scalar_tensor_tensor evacuations.  Scores S^T are k-major
bf16 (two heads row-group-paired); exp on ACT over [128,1024] PSUM
tiles, output fp8e5m2 in kc-pair layout; P@v runs DoubleRow with fp8 v
(token-major, ones column per head emitting softmax denominators in
row 64).  Denominator pairs gather via DRAM into [2,512] -> Ln+Exp(-x)
on ACT (same table set as exp; DVE reciprocal is 3.2us/call and the
recip table set would thrash) -> DRAM -> [64,512] stride-0 broadcast
-> DVE multiply.  Accumulation stays fp32 in PSUM.  fc2 runs
token-major (lhsT=gT chunks): residual add + store, no transposes.

Known limit: the ACT-paced attention leaves the PE at ~60% duty so the
HAM clock gate keeps it at 1.2 GHz there (score MMs ~427ns = 512/1.2);
dense MLP streams run warm at 2.4 GHz (DR MMs 229ns).  Warm-up bursts
+ tiny-MM heartbeats do NOT hold K=8/8 (micro-idles re-throttle) -
tried and reverted.  NOTE: exec time is bimodal (~315-321us fast mode,
~377us occasional slow mode, code-independent) - judge changes on 2-3
runs, never one.
"""

import concourse.bass as bass
import concourse.mybir as mybir
from concourse.masks import make_identity

AF = mybir.ActivationFunctionType
ALU = mybir.AluOpType
FP32 = mybir.dt.float32
BF16 = mybir.dt.bfloat16
FP8 = mybir.dt.float8e4
FP8E5 = mybir.dt.float8e5
U8 = mybir.dt.uint8
DR = mybir.MatmulPerfMode.DoubleRow
# Schraudolph constant: e4m3 bits of exp(x) = round(x*8/ln2 + 56)
SCH_A = 8.0 / 0.6931471805599453
SCH_B = 56.0

N, C, H, HD, HID = 1024, 768, 12, 64, 4 * 768
P = 128
TOK = N // P  # 8 token chunks
CT = C // P  # 6 channel chunks
CP = CT // 2  # 3 channel pairs (K=256 DoubleRow passes)
HIDT = HID // P  # 24 hidden chunks
HIDP = HIDT // 2  # 12 hidden pairs
EPS = 1e-5
SCALE = HD ** (-0.5)
# fp8 weight pre-scaling: U(-1/sqrt(fan_in), ..) weights sit below
# e4m3's normal range (2^-6); scale up before the cast, descale via
# existing free op parameters (exp scale, gelu input scale,
# scalar_tensor_tensor evacuations).
WS_QKV = 16.0
WS_PROJ = 16.0
WS_FC1 = 16.0
WS_FC2 = 32.0


def build(nc: bass.Bass, tc, with_b_proj=True, with_b_fc2=True, with_b_fc1=True):
    ctx_lp = nc.allow_low_precision(
        reason="fp8 DoubleRow matmuls, fp32 accum; validated vs fp32 reference"
    )
    ctx_lp.__enter__()
    x = nc.dram_tensor("x", [N, C], FP32, kind="ExternalInput").ap()
    ln1_g = nc.dram_tensor("ln1_g", [C], FP32, kind="ExternalInput").ap()
    ln1_b = nc.dram_tensor("ln1_b", [C], FP32, kind="ExternalInput").ap()
    # pair-layout fp8 weights (host-prepped):
    #   wqkv_p[j*128+p, i*2304+c] = w_qkv[256j+128i+p, c]
    w_qkv = nc.dram_tensor("wqkv_p", [CP * P, 2 * 3 * C], FP8, kind="ExternalInput").ap()
    w_proj = nc.dram_tensor("wproj_p", [CP * P, 2 * C], FP8, kind="ExternalInput").ap()
    b_proj = nc.dram_tensor("b_proj", [C], FP32, kind="ExternalInput").ap()
    ln2_g = nc.dram_tensor("ln2_g", [C], FP32, kind="ExternalInput").ap()
    ln2_b = nc.dram_tensor("ln2_b", [C], FP32, kind="ExternalInput").ap()
    w_fc1 = nc.dram_tensor("wfc1_p", [CP * P, 2 * HID], FP8, kind="ExternalInput").ap()
    b_fc1 = nc.dram_tensor("b_fc1", [HID], FP32, kind="ExternalInput").ap()
    w_fc2 = nc.dram_tensor("wfc2_p", [HIDP * P, 2 * C], FP8, kind="ExternalInput").ap()
    b_fc2 = nc.dram_tensor("b_fc2", [C], FP32, kind="ExternalInput").ap()
    out = nc.dram_tensor("out", [N, C], FP32, kind="ExternalOutput").ap()

    with (
        tc.tile_pool(name="singles", bufs=1) as singles,
        tc.tile_pool(name="xpool", bufs=1) as xpool,
        tc.tile_pool(name="temps", bufs=3) as temps,
        tc.tile_pool(name="stats", bufs=4) as stats,
        tc.tile_pool(name="wpool", bufs=1) as wpool,
    ):
        # --- constants -------------------------------------------------
        identB = singles.tile([P, P], BF16, tag="identB", name="identB")
        make_identity(nc, identB)
        eps_t = singles.tile([P, 1], FP32, tag="eps", name="eps")
        nc.vector.memset(eps_t, EPS)

        def col_load(vec_ap, n_ch, tag):
            """[n_ch*128] DRAM vector -> [128, n_ch] SBUF per-partition."""
            t = singles.tile([P, n_ch], FP32, tag=tag, name=tag)
            nc.sync.dma_start(out=t, in_=vec_ap.rearrange("(c p) -> p c", p=P))
            return t

        def bcast_load(vec_ap, tag):
            """[768] DRAM vector -> [128, 768] broadcast across partitions."""
            t = singles.tile([P, C], FP32, tag=tag, name=tag)
            src = bass.AP(
                tensor=vec_ap.tensor,
                offset=vec_ap.offset,
                ap=[[0, P], *vec_ap.ap],
            )
            nc.sync.dma_start(out=t, in_=src)
            return t

        # --- weights (fp8 pair layout) --------------------------------
        wqkv = [
            wpool.tile([P, 2, 3 * C], FP8, tag=f"wqkv{j}", name=f"wqkv{j}")
            for j in range(CP)
        ]
        wproj = [
            wpool.tile([P, 2, C], FP8, tag=f"wproj{j}", name=f"wproj{j}")
            for j in range(CP)
        ]
        wfc1 = [
            wpool.tile([P, 2, HID], FP8, tag=f"wfc1{j}", name=f"wfc1{j}")
            for j in range(CP)
        ]
        wfc2 = [
            wpool.tile([P, 2, C], FP8, tag=f"wfc2{j}", name=f"wfc2{j}")
            for j in range(HIDP)
        ]
        # --- load x first (LN1 is the critical path at kernel start) ---
        xt = [
            xpool.tile([P, C], FP32, tag=f"x{m}", name=f"x{m}")
            for m in range(TOK)
        ]
        # DMA issue order is Sync-queue execution order: x0-3 (LN1 can
        # start), ln1 gain/bias, qkv weights, rest of x, then the
        # late-needed weights/vectors (strided col_loads are
        # descriptor-expensive; keep them off the critical-path front).
        for m in range(4):
            nc.sync.dma_start(out=xt[m], in_=x[m * P : (m + 1) * P, :])
        g1c = col_load(ln1_g, CT, "g1c")
        b1c = col_load(ln1_b, CT, "b1c")
        for j in range(CP):
            nc.sync.dma_start(
                out=wqkv[j],
                in_=w_qkv[j * P : (j + 1) * P, :].rearrange("p (i c) -> p i c", i=2),
            )
        for m in range(4, TOK):
            nc.sync.dma_start(out=xt[m], in_=x[m * P : (m + 1) * P, :])
        bp_b = bcast_load(b_proj, "bp_b") if with_b_proj else None
        for j in range(CP):
            nc.sync.dma_start(
                out=wproj[j],
                in_=w_proj[j * P : (j + 1) * P, :].rearrange("p (i c) -> p i c", i=2),
            )
        g2c = col_load(ln2_g, CT, "g2c")
        b2c = col_load(ln2_b, CT, "b2c")
        for j in range(CP):
            nc.sync.dma_start(
                out=wfc1[j],
                in_=w_fc1[j * P : (j + 1) * P, :].rearrange("p (i c) -> p i c", i=2),
            )
        bf1c = col_load(b_fc1, HIDT, "bf1c")
        bf2_b = bcast_load(b_fc2, "bf2_b") if with_b_fc2 else None
        for j in range(HIDP):
            nc.sync.dma_start(
                out=wfc2[j],
                in_=w_fc2[j * P : (j + 1) * P, :].rearrange("p (i c) -> p i c", i=2),
            )

        def ln_normalize(src_tile):
            """token-major [128, 768] -> bf16 normalized (x-mu)*rstd."""
            st = stats.tile([P, 3, 6], FP32, tag="bnst", name="bnst")
            src3 = src_tile.rearrange("p (s d) -> p s d", s=3)
            for s in range(3):
                nc.vector.bn_stats(out=st[:, s, :], in_=src3[:, s, :])
            mv = stats.tile([P, 2], FP32, tag="bnmv", name="bnmv")
            nc.vector.bn_aggr(out=mv, in_=st)
            # rstd = exp(-0.5*ln(var+eps)); Ln+Exp share the exp table set,
            # so LN never forces an ACT table switch (Sqrt would).
            lnv = stats.tile([P, 1], FP32, tag="bnlnv", name="bnlnv")
            nc.scalar.activation(
                out=lnv, in_=mv[:, 1:2], func=AF.Ln, bias=eps_t, scale=1.0
            )
            rstd = stats.tile([P, 1], FP32, tag="bnrstd", name="bnrstd")
            nc.scalar.activation(out=rstd, in_=lnv, func=AF.Exp, scale=-0.5)
            # -mu*rstd so the normalize can run on ACT (free affine):
            # hn = Identity(x*rstd + (-mu*rstd))
            nmr = stats.tile([P, 1], FP32, tag="nmr", name="nmr")
            nc.vector.tensor_scalar(
                out=nmr, in0=mv[:, 0:1], scalar1=rstd, scalar2=-1.0,
                op0=ALU.mult, op1=ALU.mult,
            )
            hn = temps.tile([P, C], BF16, tag="hn", name="hn")
            nc.scalar.activation(
                out=hn, in_=src_tile, func=AF.Identity,
                scale=rstd, bias=nmr,
            )
            return hn

        def transpose_affine(hn, dstT_pairs, m, gcol, bcol, pspool, tag):
            """transpose bf16 token-major [128,768] into fp8 pair tiles'
            column m; g,b applied per-partition on ACT/DVE."""
            for c in range(CT):
                tp = pspool.tile([P, P], BF16, tag=tag, name=tag)
                nc.tensor.transpose(tp, hn[:, c * P : (c + 1) * P], identB)
                dst = dstT_pairs[c // 2][:, c % 2, m * P : (m + 1) * P]
                if c < CT // 2:
                    nc.scalar.activation(
                        out=dst, in_=tp, func=AF.Identity,
                        scale=gcol[:, c : c + 1], bias=bcol[:, c : c + 1],
                    )
                else:
                    nc.vector.tensor_scalar(
                        out=dst, in0=tp, scalar1=gcol[:, c : c + 1],
                        scalar2=bcol[:, c : c + 1], op0=ALU.mult, op1=ALU.add,
                    )

        with tc.tile_pool(name="hTpool", bufs=1) as hTpool:
            # --- LN1 + transpose -> hT pairs; fold b_proj into x -------
            hT = [
                hTpool.tile([P, 2, N], FP8, tag=f"hT{j}", name=f"hT{j}")
                for j in range(CP)
            ]
            with (
                tc.tile_pool(name="vxpool", bufs=1) as vxpool,
                tc.tile_pool(name="qkTpool", bufs=1) as qkTpool,
            ):
                # v in fp8 kc-pair layout for DoubleRow P@v: slot kc%2,
                # inner dim padded to 68 so the pair stride is 16-aligned
                vx = [
                    vxpool.tile(
                        [P, 2, H, HD + 4], FP8, tag=f"vx{kp}", name=f"vx{kp}"
                    )
                    for kp in range(TOK // 2)
                ]
                qkT = [
                    qkTpool.tile([P, N], BF16, tag=f"qkT{i}", name=f"qkT{i}")
                    for i in range(2 * CT)
                ]
                with (
                    tc.tile_pool(name="psA", bufs=4, space="PSUM") as psA,
                    tc.tile_pool(name="psQ", bufs=3, space="PSUM") as psQ,
                ):
                    def qkT_half(i, h):
                        """qkT tile i (i<6: q dims, else k dims), token
                        half h.  h=0 only needs token tiles 0-3, so it is
                        emitted mid-LN1 to fill the idle PE."""
                        col = i * P if i < CT else 3 * C // 2 + (i - CT) * P
                        ps = psQ.tile([P, 512], FP32, tag="qps", name="qps")
                        for j in range(CP):
                            nc.tensor.matmul(
                                ps,
                                lhsT=wqkv[j][:, :, col : col + P],
                                rhs=hT[j][:, :, h * 512 : (h + 1) * 512],
                                start=(j == 0),
                                stop=(j == CP - 1),
                                perf_mode=DR,
                            )
                        if h == 0:
                            # mid-LN1: DVE is the bottleneck, use ACT
                            nc.scalar.activation(
                                out=qkT[i][:, h * 512 : (h + 1) * 512],
                                in_=ps, func=AF.Identity,
                            )
                        else:
                            # post-LN1: ACT's queue gates the first
                            # scores; DVE is idle here
                            nc.vector.tensor_copy(
                                qkT[i][:, h * 512 : (h + 1) * 512], ps
                            )

                    for m in range(TOK):
                        hn = ln_normalize(xt[m])
                        transpose_affine(hn, hT, m, g1c, b1c, psA, "trA")
                        if with_b_proj:
                            nc.gpsimd.tensor_tensor(
                                out=xt[m], in0=xt[m], in1=bp_b, op=ALU.add
                            )
                        if m == 3:
                            for i in range(2 * CT):
                                qkT_half(i, 0)
                    for i in range(2 * CT):
                        qkT_half(i, 1)

                with tc.tile_pool(name="oTpool", bufs=1) as oTpool:
                    # --- per head-pair: qkT -> scores -> exp -> P@v ----
                    oT = [
                        oTpool.tile(
                            [P, 2, N], FP8, tag=f"oT{j}", name=f"oT{j}"
                        )
                        for j in range(CP)
                    ]
                    with (
                        tc.tile_pool(name="psS", bufs=3, space="PSUM") as psS,
                        tc.tile_pool(name="psO", bufs=2, space="PSUM") as psO,
                        tc.tile_pool(name="expp", bufs=1) as expp,
                        tc.tile_pool(name="attn_t", bufs=4) as attn_t,
                        tc.tile_pool(name="rsd", bufs=8, space="DRAM") as rsd,
                    ):
                        def make_v(m):
                            """v token tile via the score-psum ring."""
                            ps = psS.tile([P, 1024], FP32, tag="sps", name="vps")
                            for j in range(CP):
                                for n0, n1 in ((0, 512), (512, 768)):
                                    nc.tensor.matmul(
                                        ps[:, n0:n1],
                                        lhsT=hT[j][:, :, m * P : (m + 1) * P],
                                        rhs=wqkv[j][:, :, 2 * C + n0 : 2 * C + n1],
                                        start=(j == 0),
                                        stop=(j == CP - 1),
                                        perf_mode=DR,
                                    )
                            dst = vx[m // 2][:, m % 2, :, :]
                            nc.vector.memset(dst[:, :, HD : HD + 1], 1.0)
                            nc.vector.tensor_copy(
                                dst[:, :, 0:HD],
                                ps[:, 0:C].rearrange("p (h d) -> p h d", h=H),
                            )

                        # exp in fp8e4 (scores are in [-2.1, 2.1]; e4m3
                        # halves the quantization error vs e5m2).  Layout
                        # [P, kc%2, sub, qh*512+col] per kc-pair tile so
                        # one (kc,qh) psum tile exps into a contiguous
                        # [2,512] slice and the DoubleRow P@v reads
                        # [P, 2(kc), 512].
                        expS = [
                            expp.tile(
                                [P, 2, 2, N],
                                FP8,
                                tag=f"expS{kp}",
                                name=f"expS{kp}",
                            )
                            for kp in range(TOK // 2)
                        ]
                        EXP_SC = SCALE / (WS_QKV * WS_QKV)

                        def exp_emit(dst, sp, on_act):
                            if on_act:
                                nc.scalar.activation(
                                    out=dst, in_=sp, func=AF.Exp,
                                    scale=EXP_SC,
                                )
                            else:
                                nc.vector.tensor_scalar(
                                    out=dst.bitcast(U8), in0=sp,
                                    scalar1=SCH_A * EXP_SC,
                                    scalar2=SCH_B,
                                    op0=ALU.mult, op1=ALU.add,
                                )

                        def emit_scores(qt, kt, kc, live=None):
                            """Per q-half: one [128,1024] psum tile, two
                            row-group-paired score MMs (T0/T8 sharing the
                            tile, so the second MM carries no alloc wait
                            and the PE streams them concurrently).  exp
                            on ACT for qh0, Schraudolph-to-e4m3 bits on
                            DVE for qh1; every 4th kc sends qh1 to ACT
                            too to balance the denominator-reciprocal
                            work now on DVE."""
                            for qh in range(2):
                                sp = psS.tile(
                                    [P, 1024], FP32, tag="sps", name="sps"
                                )
                                for sub in range(2):
                                    rows = slice(sub * HD, (sub + 1) * HD)
                                    nc.tensor.matmul(
                                        sp[:, sub * 512 : (sub + 1) * 512],
                                        lhsT=kt[rows, kc * P : (kc + 1) * P],
                                        rhs=qt[rows, qh * 512 : (qh + 1) * 512],
                                        start=True,
                                        stop=True,
                                    )
                                dst = expS[kc // 2][
                                    :, kc % 2, :, qh * 512 : (qh + 1) * 512
                                ]
                                exp_emit(dst, sp, qh == 0)

                        def attn_chain_mms(hp, sub, chains, kp):
                            """one kc-pair DoubleRow step of the P@v
                            accumulation chains (both q-halves)."""
                            head = 2 * hp + sub
                            for qh in range(2):
                                nc.tensor.matmul(
                                    chains[qh][0 : HD + 1, :],
                                    lhsT=vx[kp][:, :, head, 0 : HD + 1],
                                    rhs=expS[kp][
                                        :, :, sub, qh * 512 : (qh + 1) * 512
                                    ],
                                    start=(kp == 0),
                                    stop=(kp == TOK // 2 - 1),
                                    perf_mode=DR,
                                )

                        def copy_out(chains):
                            """Numerators (+denominator row 64) to one
                            [65,1024] SBUF tile, freeing psO fast."""
                            oU = attn_t.tile(
                                [HD + 1, 1024], FP32, tag="oU", name="oU",
                                bufs=4,
                            )
                            for qh in range(2):
                                nc.vector.tensor_copy(
                                    oU[:, qh * 512 : (qh + 1) * 512],
                                    chains[qh][0 : HD + 1, :],
                                )
                            return oU

                        def denom_finish(oU, hp, sub, last=False):
                            """denominator row -> Ln -> Exp(-x) on ACT
                            (exp table set), one SBUF->SBUF stride-0
                            broadcast DMA across 64 partitions, then the
                            normalize multiply (GPSIMD mid-phase, DVE for
                            the last pair to keep the tail short)."""
                            head = 2 * hp + sub
                            lnd = attn_t.tile(
                                [1, 1024], FP32, tag="lnd", name="lnd"
                            )
                            nc.scalar.activation(
                                out=lnd, in_=oU[HD : HD + 1, :], func=AF.Ln,
                            )
                            rsf = attn_t.tile(
                                [1, 1024], FP32, tag="rsf", name="rsf"
                            )
                            nc.scalar.activation(
                                out=rsf, in_=lnd, func=AF.Exp, scale=-1.0,
                            )
                            rd = rsd.tile([1, 1024], FP32, tag="rd", name="rd")
                            nc.sync.dma_start(out=rd, in_=rsf)
                            rbs = attn_t.tile(
                                [HD, 1024], FP32, tag="rbs", name="rbs"
                            )
                            bsrc = bass.AP(
                                tensor=rd.tensor,
                                offset=rd.offset,
                                ap=[[0, HD], *rd.ap[1:]],
                            )
                            nc.sync.dma_start(out=rbs, in_=bsrc)
                            eng = nc.vector if last else nc.gpsimd
                            for qh in range(2):
                                dst = oT[head // 4][
                                    (head % 2) * HD : (head % 2 + 1) * HD,
                                    (head // 2) % 2,
                                    qh * 512 : (qh + 1) * 512,
                                ]
                                eng.tensor_tensor(
                                    out=dst,
                                    in0=oU[0:HD, qh * 512 : (qh + 1) * 512],
                                    in1=rbs[:, qh * 512 : (qh + 1) * 512],
                                    op=ALU.mult,
                                )

                        for hp in range(H // 2):
                            qt, kt = qkT[hp], qkT[CT + hp]
                            # kc-skewed pipeline: scores/exp run two steps
                            # ahead of the sub0 P@v chains so ACT (exp)
                            # stays saturated; sub1 chains + the next
                            # pair's first scores fill the pair boundary.
                            # v production (pair 0 only) rides the same
                            # psum ring, two steps ahead of use.
                            if hp == 0:
                                emit_scores(qt, kt, 0)
                                emit_scores(qt, kt, 1)
                                make_v(0)
                                make_v(1)
                            c0 = {}
                            for qh in range(2):
                                c0[qh] = psO.tile(
                                    [P, 512], FP32, tag="ops", name="ops"
                                )
                            for kc in range(TOK):
                                if hp == 0 and kc + 2 < TOK:
                                    make_v(kc + 2)
                                if kc + 2 < TOK:
                                    emit_scores(qt, kt, kc + 2, live=c0)
                                if kc % 2 == 1:
                                    attn_chain_mms(hp, 0, c0, kc // 2)
                            oU0 = copy_out(c0)
                            c1 = {}
                            for qh in range(2):
                                c1[qh] = psO.tile(
                                    [P, 512], FP32, tag="ops", name="ops"
                                )
                            attn_chain_mms(hp, 1, c1, 0)
                            if hp + 1 < H // 2:
                                # next pair's first scores keep ACT fed
                                # across the boundary (their exps only
                                # overwrite kc-pair 0, already consumed)
                                # and hide the denominator gather latency
                                emit_scores(qkT[hp + 1], qkT[CT + hp + 1], 0, live=c1)
                                emit_scores(qkT[hp + 1], qkT[CT + hp + 1], 1, live=c1)
                            denom_finish(oU0, hp, 0, last=(hp == H // 2 - 1))
                            for kp in range(1, TOK // 2):
                                attn_chain_mms(hp, 1, c1, kp)
                            oU1 = copy_out(c1)
                            denom_finish(oU1, hp, 1, last=(hp == H // 2 - 1))

                    # --- proj + residual + LN2, interleaved per token
                    # tile so LN2 (DVE) overlaps proj (PE) --------------
                    x1t = xt  # x tiles become x1 = x (+ b_proj) + attn
                    with tc.tile_pool(name="gTpool", bufs=1) as gTpool:
                      gT = [
                          gTpool.tile([P, 2, N], FP8, tag=f"gT{j}", name=f"gT{j}")
                          for j in range(HIDP)
                      ]
                      with tc.tile_pool(name="h2Tpool", bufs=1) as h2Tpool:
                        h2T = [
                            h2Tpool.tile([P, 2, N], FP8, tag=f"h2T{j}", name=f"h2T{j}")
                            for j in range(CP)
                        ]
                        with (
                            tc.tile_pool(name="psP", bufs=3, space="PSUM") as psP,
                            tc.tile_pool(name="psT2", bufs=2, space="PSUM") as psT2,
                        ):
                            for m in range(TOK):
                                ps = psP.tile([P, C], FP32, tag="pps", name="pps")
                                for j in range(CP):
                                    for n0, n1 in ((0, 512), (512, 768)):
                                        nc.tensor.matmul(
                                            ps[:, n0:n1],
                                            lhsT=oT[j][:, :, m * P : (m + 1) * P],
                                            rhs=wproj[j][:, :, n0:n1],
                                            start=(j == 0),
                                            stop=(j == CP - 1),
                                            perf_mode=DR,
                                        )
                                nc.vector.scalar_tensor_tensor(
                                    out=xt[m], in0=ps,
                                    scalar=1.0 / (WS_QKV * WS_PROJ),
                                    in1=xt[m],
                                    op0=ALU.mult, op1=ALU.add,
                                )
                                hn = ln_normalize(x1t[m])
                                transpose_affine(hn, h2T, m, g2c, b2c, psT2, "trB")
                                if with_b_fc2:
                                    nc.gpsimd.tensor_tensor(
                                        out=x1t[m], in0=x1t[m], in1=bf2_b, op=ALU.add
                                    )

                        # --- fc1 + gelu -> gT pairs ------------------------
                        # h outer: the h=0 half only needs LN2 of
                        # token tiles 0-3, so it starts ~8us earlier
                        with tc.tile_pool(name="psU", bufs=3, space="PSUM") as psU:
                            # mh pairs share one [128,1024] psum tile so
                            # gelu runs at half the per-call overhead
                            # (bias must be per-partition-constant, so
                            # pairing needs b_fc1 == 0; else fall back);
                            # h-outer keeps the h=0 half starting early
                            # (needs LN2 of token tiles 0-3 only).
                            for h in range(2):
                                for mhp in range(HIDP):
                                    ps = psU.tile([P, 1024], FP32, tag="ups", name="ups")
                                    for i in range(2):
                                        mh = 2 * mhp + i
                                        for j in range(CP):
                                            nc.tensor.matmul(
                                                ps[:, i * 512 : (i + 1) * 512],
                                                lhsT=wfc1[j][:, :, mh * P : (mh + 1) * P],
                                                rhs=h2T[j][:, :, h * 512 : (h + 1) * 512],
                                                start=(j == 0),
                                                stop=(j == CP - 1),
                                                perf_mode=DR,
                                            )
                                    if with_b_fc1:
                                        for i in range(2):
                                            nc.scalar.activation(
                                                out=gT[mhp][:, i, h * 512 : (h + 1) * 512],
                                                in_=ps[:, i * 512 : (i + 1) * 512],
                                                func=AF.Gelu,
                                                bias=bf1c[:, 2 * mhp + i : 2 * mhp + i + 1],
                                                scale=1.0 / WS_FC1,
                                            )
                                    else:
                                        nc.scalar.activation(
                                            out=gT[mhp][:, :, h * 512 : (h + 1) * 512],
                                            in_=ps, func=AF.Gelu,
                                            scale=1.0 / WS_FC1,
                                        )

                      # --- fc2 token-major: out[m] = x1[m] + gT.T @ wfc2 -
                      with tc.tile_pool(name="psY", bufs=2, space="PSUM") as psY:
                        for m in range(TOK):
                            ps = psY.tile([P, C], FP32, tag="yps", name="yps")
                            for j in range(HIDP):
                                for n0, n1 in ((0, 512), (512, 768)):
                                    nc.tensor.matmul(
                                        ps[:, n0:n1],
                                        lhsT=gT[j][:, :, m * P : (m + 1) * P],
                                        rhs=wfc2[j][:, :, n0:n1],
                                        start=(j == 0),
                                        stop=(j == HIDP - 1),
                                        perf_mode=DR,
                                    )
                            nc.vector.scalar_tensor_tensor(
                                out=x1t[m], in0=ps, scalar=1.0 / WS_FC2,
                                in1=x1t[m], op0=ALU.mult, op1=ALU.add,
                            )
                            nc.sync.dma_start(
                                out=out[m * P : (m + 1) * P, :], in_=x1t[m]
                            )

    ctx_lp.__exit__(None, None, None)
    return out


# ---- wait splitting (walrus allows 1 sync wait/instruction) ----

"""Post-pass: this container's walrus rejects >1 sync wait per instruction.

Tile's sem-assignment freely attaches several waits to one instruction.
Peel all but the last wait onto freshly inserted NoOp instructions on the
same engine, placed immediately before the instruction in its block.
"""


def split_multi_waits(nc, max_waits: int = 1) -> int:
    n_split = 0
    for f in nc.m.functions:
        for bb in f.blocks:
            insts = list(bb.instructions)
            out = []
            for inst in insts:
                si = inst.sync_info
                waits = list(si.on_wait) if si is not None else []
                if len(waits) > max_waits:
                    n_split += 1
                    peel = waits[:-max_waits]
                    si.on_wait = waits[-max_waits:]
                    for i in range(0, len(peel), max_waits):
                        nop = mybir.InstNoOp(
                            name=f"I-waitfix-{n_split}-{i}",
                            engine=inst.engine,
                            ins=[],
                            outs=[],
                            sync_info=mybir.SyncInfo(
                                on_wait=peel[i : i + max_waits], on_update=[]
                            ),
                        )
                        nc.register_instruction(nop)
                        out.append(nop)
                out.append(inst)
            if len(out) != len(insts):
                bb.instructions[:] = out
    return n_split


# ----------------------------------------------------------------------
# SPMD entry point: full inputs in, full outputs out (8-way batch-parallel)
# ----------------------------------------------------------------------
import numpy as _np
import ml_dtypes as _mld

_N_CORES = 8
_FP32_KEYS = ["ln1_g", "ln1_b", "b_proj", "ln2_g", "ln2_b", "b_fc1", "b_fc2"]


def _pair_fp8(w):
    """[K, M] fp32 -> [K/2 * 128?, ...] pair layout: out[j*128+p, i*M+c] =
    w[256j+128i+p, c], cast to fp8e4m3."""
    K, M = w.shape
    JP = K // 256
    w8 = w.astype(_mld.float8_e4m3fn)
    w4 = w8.reshape(JP, 2, P, M).transpose(0, 2, 1, 3)  # [j, p, i, c]
    return _np.ascontiguousarray(w4).reshape(JP * P, 2 * M)


def _prep_weights(inputs):
    w = {}
    for k in _FP32_KEYS:
        w[k] = _np.ascontiguousarray(_np.asarray(inputs[k], dtype=_np.float32))
    w["wqkv_p"] = _pair_fp8(_np.asarray(inputs["w_qkv"], dtype=_np.float32) * WS_QKV)
    w["wproj_p"] = _pair_fp8(_np.asarray(inputs["w_proj"], dtype=_np.float32) * WS_PROJ)
    w["wfc1_p"] = _pair_fp8(_np.asarray(inputs["w_fc1"], dtype=_np.float32) * WS_FC1)
    w["wfc2_p"] = _pair_fp8(_np.asarray(inputs["w_fc2"], dtype=_np.float32) * WS_FC2)
    return w


def _build_program(weights):
    import concourse.tile as tile

    nc = bass.Bass("TRN2", target_bir_lowering=False, debug=False,
                   num_devices=_N_CORES)
    with tile.TileContext(nc) as tc:
        build(
            nc, tc,
            with_b_proj=bool(_np.any(weights["b_proj"])),
            with_b_fc2=bool(_np.any(weights["b_fc2"])),
            with_b_fc1=bool(_np.any(weights["b_fc1"])),
        )
    split_multi_waits(nc)
    return nc


def kernel(**inputs):
    from concourse.bass_utils import run_bass_kernel_spmd

    x = _np.ascontiguousarray(_np.asarray(inputs["x"], dtype=_np.float32))
    assert x.shape == (8, N, C), x.shape
    weights = _prep_weights(inputs)
    nc = _build_program(weights)
    in_maps = [{"x": x[b], **weights} for b in range(_N_CORES)]
    res = run_bass_kernel_spmd(nc, in_maps, list(range(_N_CORES)))
    out = _np.stack([res.results[b]["out"] for b in range(_N_CORES)])
    return out.astype(_np.float32)



# revision 28
# speedup vs baseline: 1.0593x; 1.0593x over previous
"""Self-contained Trainium2 kernel for nn_Block (dense transformer block),
8-way batch-parallel across NeuronCores.  V2: fp8e4m3 DoubleRow matmuls
for qkv/qkT/proj/fc1/fc2, token-major fc2 (no output transposes),
softmax denominators via DVE reciprocal (off the ACT critical path),
exp in [128,1024] tiles.

Per-core program: one transformer block over one batch element
x[1024, 768] -> out[1024, 768].

Layouts: token-major = tokens on partitions; feature-major = channels on
partitions.  LN runs token-major (free-dim stats via bn_stats), then
PE-transposes into feature-major with the LN gain/bias fused into the
PSUM evacuation (per-partition scale/bias APs), quantizing to fp8 in
"pair" layout [128, 2, N] where slot i holds channel rows 256j+128i+p
(DoubleRow contraction over K=256 per matmul pass).

Weights arrive in DRAM pre-cast to fp8e4m3 in matching pair layout,
pre-scaled x16 (x32 for fc2) so U(-1/sqrt(fan_in),..) values clear
e4m3's subnormal cutoff; descales fold into exp's scale, gelu's input
scale, and scalar_tensor_tensor evacuations.  Scores S^T are k-major
bf16 (two heads row-group-paired); exp on ACT over [128,1024] PSUM
tiles, output fp8e5m2 in kc-pair layout; P@v runs DoubleRow with fp8 v
(token-major, ones column per head emitting softmax denominators in
row 64).  Denominator pairs gather via DRAM into [2,512] -> Ln+Exp(-x)
on ACT (same table set as exp; DVE reciprocal is 3.2us/call and the
recip table set would thrash) -> DRAM -> [64,512] stride-0 broadcast
-> DVE multiply.  Accumulation stays fp32 in PSUM.  fc2 runs
token-major (lhsT=gT chunks): residual add + store, no transposes.

Known limit: the ACT-paced attention leaves the PE at ~60% duty so the
HAM clock gate keeps it at 1.2 GHz there (score MMs ~427ns = 512/1.2);
dense MLP streams run warm at 2.4 GHz (DR MMs 229ns).  Warm-up bursts
+ tiny-MM heartbeats do NOT hold K=8/8 (micro-idles re-throttle) -
tried and reverted.  NOTE: exec time is bimodal (~315-321us fast mode,
~377us occasional slow mode, code-independent) - judge changes on 2-3
runs, never one.
"""

import concourse.bass as bass
import concourse.mybir as mybir
from concourse.masks import make_identity

AF = mybir.ActivationFunctionType
ALU = mybir.AluOpType
FP32 = mybir.dt.float32
BF16 = mybir.dt.bfloat16
FP8 = mybir.dt.float8e4
FP8E5 = mybir.dt.float8e5
U8 = mybir.dt.uint8
DR = mybir.MatmulPerfMode.DoubleRow
# Schraudolph constant: e4m3 bits of exp(x) = round(x*8/ln2 + 56)
SCH_A = 8.0 / 0.6931471805599453
SCH_B = 56.0

N, C, H, HD, HID = 1024, 768, 12, 64, 4 * 768
P = 128
TOK = N // P  # 8 token chunks
CT = C // P  # 6 channel chunks
CP = CT // 2  # 3 channel pairs (K=256 DoubleRow passes)
HIDT = HID // P  # 24 hidden chunks
HIDP = HIDT // 2  # 12 hidden pairs
EPS = 1e-5
SCALE = HD ** (-0.5)
# fp8 weight pre-scaling: U(-1/sqrt(fan_in), ..) weights sit below
# e4m3's normal range (2^-6); scale up before the cast, descale via
# existing free op parameters (exp scale, gelu input scale,
# scalar_tensor_tensor evacuations).
WS_QKV = 16.0
WS_PROJ = 16.0
WS_FC1 = 16.0
WS_FC2 = 32.0


def build(nc: bass.Bass, tc, with_b_proj=True, with_b_fc2=True, with_b_fc1=True):
    ctx_lp = nc.allow_low_precision(
        reason="fp8 DoubleRow matmuls, fp32 accum; validated vs fp32 reference"
    )
    ctx_lp.__enter__()
    x = nc.dram_tensor("x", [N, C], FP32, kind="ExternalInput").ap()
    ln1_g = nc.dram_tensor("ln1_g", [C], FP32, kind="ExternalInput").ap()
    ln1_b = nc.dram_tensor("ln1_b", [C], FP32, kind="ExternalInput").ap()
    # pair-layout fp8 weights (host-prepped):
    #   wqkv_p[j*128+p, i*2304+c] = w_qkv[256j+128i+p, c]
    w_qkv = nc.dram_tensor("wqkv_p", [CP * P, 2 * 3 * C], FP8, kind="ExternalInput").ap()
    w_proj = nc.dram_tensor("wproj_p", [CP * P, 2 * C], FP8, kind="ExternalInput").ap()
    b_proj = nc.dram_tensor("b_proj", [C], FP32, kind="ExternalInput").ap()
    ln2_g = nc.dram_tensor("ln2_g", [C], FP32, kind="ExternalInput").ap()
    ln2_b = nc.dram_tensor("ln2_b", [C], FP32, kind="ExternalInput").ap()
    w_fc1 = nc.dram_tensor("wfc1_p", [CP * P, 2 * HID], FP8, kind="ExternalInput").ap()
    b_fc1 = nc.dram_tensor("b_fc1", [HID], FP32, kind="ExternalInput").ap()
    w_fc2 = nc.dram_tensor("wfc2_p", [HIDP * P, 2 * C], FP8, kind="ExternalInput").ap()
    b_fc2 = nc.dram_tensor("b_fc2", [C], FP32, kind="ExternalInput").ap()
    out = nc.dram_tensor("out", [N, C], FP32, kind="ExternalOutput").ap()

    with (
        tc.tile_pool(name="singles", bufs=1) as singles,
        tc.tile_pool(name="xpool", bufs=1) as xpool,
        tc.tile_pool(name="temps", bufs=3) as temps,
        tc.tile_pool(name="stats", bufs=4) as stats,
        tc.tile_pool(name="wpool", bufs=1) as wpool,
    ):
        # --- constants -------------------------------------------------
        identB = singles.tile([P, P], BF16, tag="identB", name="identB")
        make_identity(nc, identB)
        eps_t = singles.tile([P, 1], FP32, tag="eps", name="eps")
        nc.vector.memset(eps_t, EPS)

        def col_load(vec_ap, n_ch, tag):
            """[n_ch*128] DRAM vector -> [128, n_ch] SBUF per-partition."""
            t = singles.tile([P, n_ch], FP32, tag=tag, name=tag)
            nc.sync.dma_start(out=t, in_=vec_ap.rearrange("(c p) -> p c", p=P))
            return t

        def bcast_load(vec_ap, tag):
            """[768] DRAM vector -> [128, 768] broadcast across partitions."""
            t = singles.tile([P, C], FP32, tag=tag, name=tag)
            src = bass.AP(
                tensor=vec_ap.tensor,
                offset=vec_ap.offset,
                ap=[[0, P], *vec_ap.ap],
            )
            nc.sync.dma_start(out=t, in_=src)
            return t

        # --- weights (fp8 pair layout) --------------------------------
        wqkv = [
            wpool.tile([P, 2, 3 * C], FP8, tag=f"wqkv{j}", name=f"wqkv{j}")
            for j in range(CP)
        ]
        wproj = [
            wpool.tile([P, 2, C], FP8, tag=f"wproj{j}", name=f"wproj{j}")
            for j in range(CP)
        ]
        wfc1 = [
            wpool.tile([P, 2, HID], FP8, tag=f"wfc1{j}", name=f"wfc1{j}")
            for j in range(CP)
        ]
        wfc2 = [
            wpool.tile([P, 2, C], FP8, tag=f"wfc2{j}", name=f"wfc2{j}")
            for j in range(HIDP)
        ]
        # --- load x first (LN1 is the critical path at kernel start) ---
        xt = [
            xpool.tile([P, C], FP32, tag=f"x{m}", name=f"x{m}")
            for m in range(TOK)
        ]
        # DMA issue order is Sync-queue execution order: x0-3 (LN1 can
        # start), ln1 gain/bias, qkv weights, rest of x, then the
        # late-needed weights/vectors (strided col_loads are
        # descriptor-expensive; keep them off the critical-path front).
        for m in range(4):
            nc.sync.dma_start(out=xt[m], in_=x[m * P : (m + 1) * P, :])
        g1c = col_load(ln1_g, CT, "g1c")
        b1c = col_load(ln1_b, CT, "b1c")
        for j in range(CP):
            nc.sync.dma_start(
                out=wqkv[j],
                in_=w_qkv[j * P : (j + 1) * P, :].rearrange("p (i c) -> p i c", i=2),
            )
        for m in range(4, TOK):
            nc.sync.dma_start(out=xt[m], in_=x[m * P : (m + 1) * P, :])
        bp_b = bcast_load(b_proj, "bp_b") if with_b_proj else None
        for j in range(CP):
            nc.sync.dma_start(
                out=wproj[j],
                in_=w_proj[j * P : (j + 1) * P, :].rearrange("p (i c) -> p i c", i=2),
            )
        g2c = col_load(ln2_g, CT, "g2c")
        b2c = col_load(ln2_b, CT, "b2c")
        for j in range(CP):
            nc.sync.dma_start(
                out=wfc1[j],
                in_=w_fc1[j * P : (j + 1) * P, :].rearrange("p (i c) -> p i c", i=2),
            )
        bf1c = col_load(b_fc1, HIDT, "bf1c")
        bf2_b = bcast_load(b_fc2, "bf2_b") if with_b_fc2 else None
        for j in range(HIDP):
            nc.sync.dma_start(
                out=wfc2[j],
                in_=w_fc2[j * P : (j + 1) * P, :].rearrange("p (i c) -> p i c", i=2),
            )

        def ln_normalize(src_tile):
            """token-major [128, 768] -> bf16 normalized (x-mu)*rstd."""
            st = stats.tile([P, 3, 6], FP32, tag="bnst", name="bnst")
            src3 = src_tile.rearrange("p (s d) -> p s d", s=3)
            for s in range(3):
                nc.vector.bn_stats(out=st[:, s, :], in_=src3[:, s, :])
            mv = stats.tile([P, 2], FP32, tag="bnmv", name="bnmv")
            nc.vector.bn_aggr(out=mv, in_=st)
            # rstd = exp(-0.5*ln(var+eps)); Ln+Exp share the exp table set,
            # so LN never forces an ACT table switch (Sqrt would).
            lnv = stats.tile([P, 1], FP32, tag="bnlnv", name="bnlnv")
            nc.scalar.activation(
                out=lnv, in_=mv[:, 1:2], func=AF.Ln, bias=eps_t, scale=1.0
            )
            rstd = stats.tile([P, 1], FP32, tag="bnrstd", name="bnrstd")
            nc.scalar.activation(out=rstd, in_=lnv, func=AF.Exp, scale=-0.5)
            # -mu*rstd so the normalize can run on ACT (free affine):
            # hn = Identity(x*rstd + (-mu*rstd))
            nmr = stats.tile([P, 1], FP32, tag="nmr", name="nmr")
            nc.vector.tensor_scalar(
                out=nmr, in0=mv[:, 0:1], scalar1=rstd, scalar2=-1.0,
                op0=ALU.mult, op1=ALU.mult,
            )
            hn = temps.tile([P, C], BF16, tag="hn", name="hn")
            nc.scalar.activation(
                out=hn, in_=src_tile, func=AF.Identity,
                scale=rstd, bias=nmr,
            )
            return hn

        def transpose_affine(hn, dstT_pairs, m, gcol, bcol, pspool, tag):
            """transpose bf16 token-major [128,768] into fp8 pair tiles'
            column m; g,b applied per-partition on ACT/DVE."""
            for c in range(CT):
                tp = pspool.tile([P, P], BF16, tag=tag, name=tag)
                nc.tensor.transpose(tp, hn[:, c * P : (c + 1) * P], identB)
                dst = dstT_pairs[c // 2][:, c % 2, m * P : (m + 1) * P]
                if c < CT // 2:
                    nc.scalar.activation(
                        out=dst, in_=tp, func=AF.Identity,
                        scale=gcol[:, c : c + 1], bias=bcol[:, c : c + 1],
                    )
                else:
                    nc.vector.tensor_scalar(
                        out=dst, in0=tp, scalar1=gcol[:, c : c + 1],
                        scalar2=bcol[:, c : c + 1], op0=ALU.mult, op1=ALU.add,
                    )

        with tc.tile_pool(name="hTpool", bufs=1) as hTpool:
            # --- LN1 + transpose -> hT pairs; fold b_proj into x -------
            hT = [
                hTpool.tile([P, 2, N], FP8, tag=f"hT{j}", name=f"hT{j}")
                for j in range(CP)
            ]
            with (
                tc.tile_pool(name="vxpool", bufs=1) as vxpool,
                tc.tile_pool(name="qkTpool", bufs=1) as qkTpool,
            ):
                # v in fp8 kc-pair layout for DoubleRow P@v: slot kc%2,
                # inner dim padded to 68 so the pair stride is 16-aligned
                vx = [
                    vxpool.tile(
                        [P, 2, H, HD + 4], FP8, tag=f"vx{kp}", name=f"vx{kp}"
                    )
                    for kp in range(TOK // 2)
                ]
                qkT = [
                    qkTpool.tile([P, N], BF16, tag=f"qkT{i}", name=f"qkT{i}")
                    for i in range(2 * CT)
                ]
                with (
                    tc.tile_pool(name="psA", bufs=4, space="PSUM") as psA,
                    tc.tile_pool(name="psQ", bufs=3, space="PSUM") as psQ,
                ):
                    def qkT_half(i, h):
                        """qkT tile i (i<6: q dims, else k dims), token
                        half h.  h=0 only needs token tiles 0-3, so it is
                        emitted mid-LN1 to fill the idle PE."""
                        col = i * P if i < CT else 3 * C // 2 + (i - CT) * P
                        ps = psQ.tile([P, 512], FP32, tag="qps", name="qps")
                        for j in range(CP):
                            nc.tensor.matmul(
                                ps,
                                lhsT=wqkv[j][:, :, col : col + P],
                                rhs=hT[j][:, :, h * 512 : (h + 1) * 512],
                                start=(j == 0),
                                stop=(j == CP - 1),
                                perf_mode=DR,
                            )
                        if h == 0:
                            # mid-LN1: DVE is the bottleneck, use ACT
                            nc.scalar.activation(
                                out=qkT[i][:, h * 512 : (h + 1) * 512],
                                in_=ps, func=AF.Identity,
                            )
                        else:
                            # post-LN1: ACT's queue gates the first
                            # scores; DVE is idle here
                            nc.vector.tensor_copy(
                                qkT[i][:, h * 512 : (h + 1) * 512], ps
                            )

                    for m in range(TOK):
                        hn = ln_normalize(xt[m])
                        transpose_affine(hn, hT, m, g1c, b1c, psA, "trA")
                        if with_b_proj:
                            nc.gpsimd.tensor_tensor(
                                out=xt[m], in0=xt[m], in1=bp_b, op=ALU.add
                            )
                        if m == 3:
                            for i in range(2 * CT):
                                qkT_half(i, 0)
                    for i in range(2 * CT):
                        qkT_half(i, 1)

                with tc.tile_pool(name="oTpool", bufs=1) as oTpool:
                    # --- per head-pair: qkT -> scores -> exp -> P@v ----
                    oT = [
                        oTpool.tile(
                            [P, 2, N], FP8, tag=f"oT{j}", name=f"oT{j}"
                        )
                        for j in range(CP)
                    ]
                    with (
                        tc.tile_pool(name="psS", bufs=3, space="PSUM") as psS,
                        tc.tile_pool(name="psO", bufs=2, space="PSUM") as psO,
                        tc.tile_pool(name="expp", bufs=1) as expp,
                        tc.tile_pool(name="attn_t", bufs=4) as attn_t,
                        tc.tile_pool(name="rsd", bufs=8, space="DRAM") as rsd,
                    ):
                        def make_v(m):
                            """v token tile via the score-psum ring."""
                            ps = psS.tile([P, 1024], FP32, tag="sps", name="vps")
                            for j in range(CP):
                                for n0, n1 in ((0, 512), (512, 768)):
                                    nc.tensor.matmul(
                                        ps[:, n0:n1],
                                        lhsT=hT[j][:, :, m * P : (m + 1) * P],
                                        rhs=wqkv[j][:, :, 2 * C + n0 : 2 * C + n1],
                                        start=(j == 0),
                                        stop=(j == CP - 1),
                                        perf_mode=DR,
                                    )
                            dst = vx[m // 2][:, m % 2, :, :]
                            nc.vector.memset(dst[:, :, HD : HD + 1], 1.0)
                            nc.vector.tensor_copy(
                                dst[:, :, 0:HD],
                                ps[:, 0:C].rearrange("p (h d) -> p h d", h=H),
                            )

                        # exp in fp8e4 (scores are in [-2.1, 2.1]; e4m3
                        # halves the quantization error vs e5m2).  Layout
                        # [P, kc%2, sub, qh*512+col] per kc-pair tile so
                        # one (kc,qh) psum tile exps into a contiguous
                        # [2,512] slice and the DoubleRow P@v reads
                        # [P, 2(kc), 512].
                        # double-buffered by head-pair parity so pair p's
                        # P@v chains (running during pair p+1's score
                        # phase) never race the new exps
                        expS2 = [
                            [
                                expp.tile(
                                    [P, 2, 2, N],
                                    FP8,
                                    tag=f"expS{par}_{kp}",
                                    name=f"expS{par}_{kp}",
                                )
                                for kp in range(TOK // 2)
                            ]
                            for par in range(2)
                        ]
                        EXP_SC = SCALE / (WS_QKV * WS_QKV)

                        def exp_emit(dst, sp, on_act):
                            if on_act:
                                nc.scalar.activation(
                                    out=dst, in_=sp, func=AF.Exp,
                                    scale=EXP_SC,
                                )
                            else:
                                nc.vector.tensor_scalar(
                                    out=dst.bitcast(U8), in0=sp,
                                    scalar1=SCH_A * EXP_SC,
                                    scalar2=SCH_B,
                                    op0=ALU.mult, op1=ALU.add,
                                )

                        def emit_scores(qt, kt, kc, par):
                            """Per q-half: one [128,1024] psum tile, two
                            row-group-paired score MMs (T0/T8 sharing the
                            tile, so the second MM carries no alloc wait
                            and the PE streams them concurrently).  exp
                            on ACT for qh0, Schraudolph-to-e4m3 bits on
                            DVE for qh1 (parallel engines)."""
                            for qh in range(2):
                                sp = psS.tile(
                                    [P, 1024], FP32, tag="sps", name="sps"
                                )
                                for sub in range(2):
                                    rows = slice(sub * HD, (sub + 1) * HD)
                                    nc.tensor.matmul(
                                        sp[:, sub * 512 : (sub + 1) * 512],
                                        lhsT=kt[rows, kc * P : (kc + 1) * P],
                                        rhs=qt[rows, qh * 512 : (qh + 1) * 512],
                                        start=True,
                                        stop=True,
                                    )
                                dst = expS2[par][kc // 2][
                                    :, kc % 2, :, qh * 512 : (qh + 1) * 512
                                ]
                                exp_emit(dst, sp, qh == 0)

                        def attn_chain_mms(hp, sub, chains, kps):
                            """kc-pair DoubleRow steps of the P@v
                            accumulation chains (both q-halves); batched
                            kps cut tiling-mode switches."""
                            head = 2 * hp + sub
                            for kp in kps:
                                for qh in range(2):
                                    nc.tensor.matmul(
                                        chains[qh][0 : HD + 1, :],
                                        lhsT=vx[kp][:, :, head, 0 : HD + 1],
                                        rhs=expS2[hp % 2][kp][
                                            :, :, sub, qh * 512 : (qh + 1) * 512
                                        ],
                                        start=(kp == 0),
                                        stop=(kp == TOK // 2 - 1),
                                        perf_mode=DR,
                                    )

                        def copy_out(chains):
                            """Numerators (+denominator row 64) to one
                            [65,1024] SBUF tile, freeing psO fast."""
                            oU = attn_t.tile(
                                [HD + 1, 1024], FP32, tag="oU", name="oU",
                                bufs=4,
                            )
                            for qh in range(2):
                                nc.vector.tensor_copy(
                                    oU[:, qh * 512 : (qh + 1) * 512],
                                    chains[qh][0 : HD + 1, :],
                                )
                            return oU

                        def bcast64(src_row):
                            """[1,1024] DRAM row -> [64,1024] stride-0."""
                            rbs = attn_t.tile(
                                [HD, 1024], FP32, tag="rbs", name="rbs",
                                bufs=2,
                            )
                            bsrc = bass.AP(
                                tensor=src_row.tensor,
                                offset=src_row.offset,
                                ap=[[0, HD], *src_row.ap[1:]],
                            )
                            nc.sync.dma_start(out=rbs, in_=bsrc)
                            return rbs

                        def norm_mult(oU, rbs, head, eng):
                            """oT[head] = oU * recip-denominator, one
                            [64,1024] op (both q-halves)."""
                            dst = oT[head // 4][
                                (head % 2) * HD : (head % 2 + 1) * HD,
                                (head // 2) % 2,
                                :,
                            ]
                            eng.tensor_tensor(
                                out=dst, in0=oU[0:HD, :], in1=rbs,
                                op=ALU.mult,
                            )

                        def denom_single(oU, hp, sub, eng):
                            """denominator row -> Ln -> Exp(-x) on ACT
                            (exp table set) -> DRAM -> [64,1024]
                            stride-0 broadcast -> normalize multiply."""
                            lnd = attn_t.tile(
                                [1, 1024], FP32, tag="lnd", name="lnd",
                                bufs=1,
                            )
                            nc.scalar.activation(
                                out=lnd, in_=oU[HD : HD + 1, :], func=AF.Ln,
                            )
                            rsf = attn_t.tile(
                                [1, 1024], FP32, tag="rsf", name="rsf",
                                bufs=1,
                            )
                            nc.scalar.activation(
                                out=rsf, in_=lnd, func=AF.Exp, scale=-1.0,
                            )
                            rd = rsd.tile([1, 1024], FP32, tag="rd", name="rd")
                            nc.sync.dma_start(out=rd, in_=rsf)
                            norm_mult(oU, bcast64(rd[0:1, :]), 2 * hp + sub, eng)

                        def denom_batch(oU0, oU1, hp):
                            """both subs' denominator rows gathered into
                            one [2,1024] tile (SBUF->SBUF DMA) -> one Ln
                            + one Exp -> DRAM -> two broadcasts."""
                            dn = attn_t.tile(
                                [2, 1024], FP32, tag="dn", name="dn", bufs=1
                            )
                            nc.sync.dma_start(
                                out=dn[0:1, :], in_=oU0[HD : HD + 1, :]
                            )
                            nc.sync.dma_start(
                                out=dn[1:2, :], in_=oU1[HD : HD + 1, :]
                            )
                            nc.scalar.activation(out=dn, in_=dn, func=AF.Ln)
                            rsf = attn_t.tile(
                                [2, 1024], FP32, tag="rsf2", name="rsf2",
                                bufs=1,
                            )
                            nc.scalar.activation(
                                out=rsf, in_=dn, func=AF.Exp, scale=-1.0,
                            )
                            rd = rsd.tile([2, 1024], FP32, tag="rd2", name="rd2")
                            nc.sync.dma_start(out=rd, in_=rsf)
                            return rd

                        def alloc_chains():
                            return {
                                qh: psO.tile(
                                    [P, 512], FP32, tag="ops", name="ops"
                                )
                                for qh in range(2)
                            }

                        # --- cross-pair pipeline: pair p's P@v chains,
                        # numerator copies and denominators all run
                        # inside pair p+1's score/exp phase (which is
                        # exp-paced, leaving PE and Sync slack).  Pair 0
                        # interleaves v production instead; the last
                        # pair starts its own sub0 chains early and
                        # finishes with a short DVE tail. -------------
                        HP = H // 2
                        carry = None  # (c0, c1, oU0, oU1) of prev pair
                        for hp in range(HP):
                            qt, kt = qkT[hp], qkT[CT + hp]
                            lastp = hp == HP - 1
                            pv = {}
                            for kc in range(TOK):
                                emit_scores(qt, kt, kc, hp % 2)
                                if hp == 0:
                                    make_v(kc)
                                    continue
                                p = hp - 1
                                if kc == 0:
                                    pv["c0"] = alloc_chains()
                                    attn_chain_mms(p, 0, pv["c0"], (0, 1))
                                elif kc == 1:
                                    attn_chain_mms(p, 0, pv["c0"], (2, 3))
                                elif kc == 2:
                                    pv["oU0"] = copy_out(pv["c0"])
                                elif kc == 3:
                                    pv["c1"] = alloc_chains()
                                    attn_chain_mms(p, 1, pv["c1"], (0, 1))
                                elif kc == 4:
                                    attn_chain_mms(p, 1, pv["c1"], (2, 3))
                                elif kc == 5:
                                    pv["oU1"] = copy_out(pv["c1"])
                                elif kc == 6:
                                    pv["rd"] = denom_batch(
                                        pv["oU0"], pv["oU1"], p
                                    )
                                    if lastp:
                                        pv["mine0"] = alloc_chains()
                                        attn_chain_mms(
                                            hp, 0, pv["mine0"], (0, 1)
                                        )
                                elif kc == 7:
                                    norm_mult(
                                        pv["oU0"], bcast64(pv["rd"][0:1, :]),
                                        2 * p, nc.gpsimd,
                                    )
                                    norm_mult(
                                        pv["oU1"], bcast64(pv["rd"][1:2, :]),
                                        2 * p + 1, nc.gpsimd,
                                    )
                            if hp == 0:
                                continue
                            if not lastp:
                                carry = pv
                                continue
                            # --- last pair tail ------------------------
                            c0 = pv["mine0"]
                            attn_chain_mms(hp, 0, c0, (2, 3))
                            oU0 = copy_out(c0)
                            c1 = alloc_chains()
                            attn_chain_mms(hp, 1, c1, (0, 1, 2, 3))
                            denom_single(oU0, hp, 0, nc.vector)
                            oU1 = copy_out(c1)
                            denom_single(oU1, hp, 1, nc.vector)

                    # --- proj + residual + LN2, interleaved per token
                    # tile so LN2 (DVE) overlaps proj (PE) --------------
                    x1t = xt  # x tiles become x1 = x (+ b_proj) + attn
                    with tc.tile_pool(name="gTpool", bufs=1) as gTpool:
                      gT = [
                          gTpool.tile([P, 2, N], FP8, tag=f"gT{j}", name=f"gT{j}")
                          for j in range(HIDP)
                      ]
                      with tc.tile_pool(name="h2Tpool", bufs=1) as h2Tpool:
                        h2T = [
                            h2Tpool.tile([P, 2, N], FP8, tag=f"h2T{j}", name=f"h2T{j}")
                            for j in range(CP)
                        ]
                        with (
                            tc.tile_pool(name="psP", bufs=3, space="PSUM") as psP,
                            tc.tile_pool(name="psT2", bufs=2, space="PSUM") as psT2,
                        ):
                            for m in range(TOK):
                                ps = psP.tile([P, C], FP32, tag="pps", name="pps")
                                for j in range(CP):
                                    for n0, n1 in ((0, 512), (512, 768)):
                                        nc.tensor.matmul(
                                            ps[:, n0:n1],
                                            lhsT=oT[j][:, :, m * P : (m + 1) * P],
                                            rhs=wproj[j][:, :, n0:n1],
                                            start=(j == 0),
                                            stop=(j == CP - 1),
                                            perf_mode=DR,
                                        )
                                nc.vector.scalar_tensor_tensor(
                                    out=xt[m], in0=ps,
                                    scalar=1.0 / (WS_QKV * WS_PROJ),
                                    in1=xt[m],
                                    op0=ALU.mult, op1=ALU.add,
                                )
                                hn = ln_normalize(x1t[m])
                                transpose_affine(hn, h2T, m, g2c, b2c, psT2, "trB")
                                if with_b_fc2:
                                    nc.gpsimd.tensor_tensor(
                                        out=x1t[m], in0=x1t[m], in1=bf2_b, op=ALU.add
                                    )

                        # --- fc1 + gelu -> gT pairs ------------------------
                        # h outer: the h=0 half only needs LN2 of
                        # token tiles 0-3, so it starts ~8us earlier
                        with tc.tile_pool(name="psU", bufs=3, space="PSUM") as psU:
                            # mh pairs share one [128,1024] psum tile so
                            # gelu runs at half the per-call overhead
                            # (bias must be per-partition-constant, so
                            # pairing needs b_fc1 == 0; else fall back);
                            # h-outer keeps the h=0 half starting early
                            # (needs LN2 of token tiles 0-3 only).
                            for h in range(2):
                                for mhp in range(HIDP):
                                    ps = psU.tile([P, 1024], FP32, tag="ups", name="ups")
                                    for i in range(2):
                                        mh = 2 * mhp + i
                                        for j in range(CP):
                                            nc.tensor.matmul(
                                                ps[:, i * 512 : (i + 1) * 512],
                                                lhsT=wfc1[j][:, :, mh * P : (mh + 1) * P],
                                                rhs=h2T[j][:, :, h * 512 : (h + 1) * 512],
                                                start=(j == 0),
                                                stop=(j == CP - 1),
                                                perf_mode=DR,
                                            )
                                    if with_b_fc1:
                                        for i in range(2):
                                            nc.scalar.activation(
                                                out=gT[mhp][:, i, h * 512 : (h + 1) * 512],
                                                in_=ps[:, i * 512 : (i + 1) * 512],
                                                func=AF.Gelu,
                                                bias=bf1c[:, 2 * mhp + i : 2 * mhp + i + 1],
                                                scale=1.0 / WS_FC1,
                                            )
                                    else:
                                        nc.scalar.activation(
                                            out=gT[mhp][:, :, h * 512 : (h + 1) * 512],
                                            in_=ps, func=AF.Gelu,
                                            scale=1.0 / WS_FC1,
                                        )

                      # --- fc2 token-major: out[m] = x1[m] + gT.T @ wfc2 -
                      with tc.tile_pool(name="psY", bufs=2, space="PSUM") as psY:
                        for m in range(TOK):
                            ps = psY.tile([P, C], FP32, tag="yps", name="yps")
                            for j in range(HIDP):
                                for n0, n1 in ((0, 512), (512, 768)):
                                    nc.tensor.matmul(
                                        ps[:, n0:n1],
                                        lhsT=gT[j][:, :, m * P : (m + 1) * P],
                                        rhs=wfc2[j][:, :, n0:n1],
                                        start=(j == 0),
                                        stop=(j == HIDP - 1),
                                        perf_mode=DR,
                                    )
                            nc.vector.scalar_tensor_tensor(
                                out=x1t[m], in0=ps, scalar=1.0 / WS_FC2,
                                in1=x1t[m], op0=ALU.mult, op1=ALU.add,
                            )
                            nc.sync.dma_start(
                                out=out[m * P : (m + 1) * P, :], in_=x1t[m]
                            )

    ctx_lp.__exit__(None, None, None)
    return out


# ---- wait splitting (walrus allows 1 sync wait/instruction) ----

"""Post-pass: this container's walrus rejects >1 sync wait per instruction.

Tile's sem-assignment freely attaches several waits to one instruction.
Peel all but the last wait onto freshly inserted NoOp instructions on the
same engine, placed immediately before the instruction in its block.
"""


def split_multi_waits(nc, max_waits: int = 1) -> int:
    n_split = 0
    for f in nc.m.functions:
        for bb in f.blocks:
            insts = list(bb.instructions)
            out = []
            for inst in insts:
                si = inst.sync_info
                waits = list(si.on_wait) if si is not None else []
                if len(waits) > max_waits:
                    n_split += 1
                    peel = waits[:-max_waits]
                    si.on_wait = waits[-max_waits:]
                    for i in range(0, len(peel), max_waits):
                        nop = mybir.InstNoOp(
                            name=f"I-waitfix-{n_split}-{i}",
                            engine=inst.engine,
                            ins=[],
                            outs=[],
                            sync_info=mybir.SyncInfo(
                                on_wait=peel[i : i + max_waits], on_update=[]
                            ),
                        )
                        nc.register_instruction(nop)
                        out.append(nop)
                out.append(inst)
            if len(out) != len(insts):
                bb.instructions[:] = out
    return n_split


# ----------------------------------------------------------------------
# SPMD entry point: full inputs in, full outputs out (8-way batch-parallel)
# ----------------------------------------------------------------------
import numpy as _np
import ml_dtypes as _mld

_N_CORES = 8
_FP32_KEYS = ["ln1_g", "ln1_b", "b_proj", "ln2_g", "ln2_b", "b_fc1", "b_fc2"]


def _pair_fp8(w):
    """[K, M] fp32 -> [K/2 * 128?, ...] pair layout: out[j*128+p, i*M+c] =
    w[256j+128i+p, c], cast to fp8e4m3."""
    K, M = w.shape
    JP = K // 256
    w8 = w.astype(_mld.float8_e4m3fn)
    w4 = w8.reshape(JP, 2, P, M).transpose(0, 2, 1, 3)  # [j, p, i, c]
    return _np.ascontiguousarray(w4).reshape(JP * P, 2 * M)


def _prep_weights(inputs):
    w = {}
    for k in _FP32_KEYS:
        w[k] = _np.ascontiguousarray(_np.asarray(inputs[k], dtype=_np.float32))
    w["wqkv_p"] = _pair_fp8(_np.asarray(inputs["w_qkv"], dtype=_np.float32) * WS_QKV)
    w["wproj_p"] = _pair_fp8(_np.asarray(inputs["w_proj"], dtype=_np.float32) * WS_PROJ)
    w["wfc1_p"] = _pair_fp8(_np.asarray(inputs["w_fc1"], dtype=_np.float32) * WS_FC1)
    w["wfc2_p"] = _pair_fp8(_np.asarray(inputs["w_fc2"], dtype=_np.float32) * WS_FC2)
    return w


def _build_program(weights):
    import concourse.tile as tile

    nc = bass.Bass("TRN2", target_bir_lowering=False, debug=False,
                   num_devices=_N_CORES)
    with tile.TileContext(nc) as tc:
        build(
            nc, tc,
            with_b_proj=bool(_np.any(weights["b_proj"])),
            with_b_fc2=bool(_np.any(weights["b_fc2"])),
            with_b_fc1=bool(_np.any(weights["b_fc1"])),
        )
    split_multi_waits(nc)
    return nc


def kernel(**inputs):
    from concourse.bass_utils import run_bass_kernel_spmd

    x = _np.ascontiguousarray(_np.asarray(inputs["x"], dtype=_np.float32))
    assert x.shape == (8, N, C), x.shape
    weights = _prep_weights(inputs)
    nc = _build_program(weights)
    in_maps = [{"x": x[b], **weights} for b in range(_N_CORES)]
    res = run_bass_kernel_spmd(nc, in_maps, list(range(_N_CORES)))
    out = _np.stack([res.results[b]["out"] for b in range(_N_CORES)])
    return out.astype(_np.float32)



# revision 29
# speedup vs baseline: 1.0630x; 1.0035x over previous
"""Self-contained Trainium2 kernel for nn_Block (dense transformer block),
8-way batch-parallel across NeuronCores.  V2: fp8e4m3 DoubleRow matmuls
for qkv/qkT/proj/fc1/fc2, token-major fc2 (no output transposes),
softmax denominators via DVE reciprocal (off the ACT critical path),
exp in [128,1024] tiles.

Per-core program: one transformer block over one batch element
x[1024, 768] -> out[1024, 768].

Layouts: token-major = tokens on partitions; feature-major = channels on
partitions.  LN runs token-major (free-dim stats via bn_stats), then
PE-transposes into feature-major with the LN gain/bias fused into the
PSUM evacuation (per-partition scale/bias APs), quantizing to fp8 in
"pair" layout [128, 2, N] where slot i holds channel rows 256j+128i+p
(DoubleRow contraction over K=256 per matmul pass).

Weights arrive in DRAM pre-cast to fp8e4m3 in matching pair layout,
pre-scaled x16 (x32 for fc2) so U(-1/sqrt(fan_in),..) values clear
e4m3's subnormal cutoff; descales fold into exp's scale, gelu's input
scale, and scalar_tensor_tensor evacuations.  Scores S^T are k-major
bf16 (two heads row-group-paired); exp on ACT over [128,1024] PSUM
tiles, output fp8e5m2 in kc-pair layout; P@v runs DoubleRow with fp8 v
(token-major, ones column per head emitting softmax denominators in
row 64).  Denominator pairs gather via DRAM into [2,512] -> Ln+Exp(-x)
on ACT (same table set as exp; DVE reciprocal is 3.2us/call and the
recip table set would thrash) -> DRAM -> [64,512] stride-0 broadcast
-> DVE multiply.  Accumulation stays fp32 in PSUM.  fc2 runs
token-major (lhsT=gT chunks): residual add + store, no transposes.

Known limit: the ACT-paced attention leaves the PE at ~60% duty so the
HAM clock gate keeps it at 1.2 GHz there (score MMs ~427ns = 512/1.2);
dense MLP streams run warm at 2.4 GHz (DR MMs 229ns).  Warm-up bursts
+ tiny-MM heartbeats do NOT hold K=8/8 (micro-idles re-throttle) -
tried and reverted.  NOTE: exec time is bimodal (~315-321us fast mode,
~377us occasional slow mode, code-independent) - judge changes on 2-3
runs, never one.
"""

import concourse.bass as bass
import concourse.mybir as mybir
from concourse.masks import make_identity

AF = mybir.ActivationFunctionType
ALU = mybir.AluOpType
FP32 = mybir.dt.float32
BF16 = mybir.dt.bfloat16
FP8 = mybir.dt.float8e4
FP8E5 = mybir.dt.float8e5
U8 = mybir.dt.uint8
DR = mybir.MatmulPerfMode.DoubleRow
# Schraudolph constant: e4m3 bits of exp(x) = round(x*8/ln2 + 56)
SCH_A = 8.0 / 0.6931471805599453
SCH_B = 56.0

N, C, H, HD, HID = 1024, 768, 12, 64, 4 * 768
P = 128
TOK = N // P  # 8 token chunks
CT = C // P  # 6 channel chunks
CP = CT // 2  # 3 channel pairs (K=256 DoubleRow passes)
HIDT = HID // P  # 24 hidden chunks
HIDP = HIDT // 2  # 12 hidden pairs
EPS = 1e-5
SCALE = HD ** (-0.5)
# fp8 weight pre-scaling: U(-1/sqrt(fan_in), ..) weights sit below
# e4m3's normal range (2^-6); scale up before the cast, descale via
# existing free op parameters (exp scale, gelu input scale,
# scalar_tensor_tensor evacuations).
WS_QKV = 16.0
WS_PROJ = 16.0
WS_FC1 = 16.0
WS_FC2 = 32.0


def build(nc: bass.Bass, tc, with_b_proj=True, with_b_fc2=True, with_b_fc1=True):
    ctx_lp = nc.allow_low_precision(
        reason="fp8 DoubleRow matmuls, fp32 accum; validated vs fp32 reference"
    )
    ctx_lp.__enter__()
    x = nc.dram_tensor("x", [N, C], FP32, kind="ExternalInput").ap()
    ln1_g = nc.dram_tensor("ln1_g", [C], FP32, kind="ExternalInput").ap()
    ln1_b = nc.dram_tensor("ln1_b", [C], FP32, kind="ExternalInput").ap()
    # pair-layout fp8 weights (host-prepped):
    #   wqkv_p[j*128+p, i*2304+c] = w_qkv[256j+128i+p, c]
    w_qkv = nc.dram_tensor("wqkv_p", [CP * P, 2 * 3 * C], FP8, kind="ExternalInput").ap()
    w_proj = nc.dram_tensor("wproj_p", [CP * P, 2 * C], FP8, kind="ExternalInput").ap()
    b_proj = nc.dram_tensor("b_proj", [C], FP32, kind="ExternalInput").ap()
    ln2_g = nc.dram_tensor("ln2_g", [C], FP32, kind="ExternalInput").ap()
    ln2_b = nc.dram_tensor("ln2_b", [C], FP32, kind="ExternalInput").ap()
    w_fc1 = nc.dram_tensor("wfc1_p", [CP * P, 2 * HID], FP8, kind="ExternalInput").ap()
    b_fc1 = nc.dram_tensor("b_fc1", [HID], FP32, kind="ExternalInput").ap()
    w_fc2 = nc.dram_tensor("wfc2_p", [HIDP * P, 2 * C], FP8, kind="ExternalInput").ap()
    b_fc2 = nc.dram_tensor("b_fc2", [C], FP32, kind="ExternalInput").ap()
    out = nc.dram_tensor("out", [N, C], FP32, kind="ExternalOutput").ap()

    with (
        tc.tile_pool(name="singles", bufs=1) as singles,
        tc.tile_pool(name="xpool", bufs=1) as xpool,
        tc.tile_pool(name="temps", bufs=3) as temps,
        tc.tile_pool(name="stats", bufs=4) as stats,
        tc.tile_pool(name="wpool", bufs=1) as wpool,
    ):
        # --- constants -------------------------------------------------
        identB = singles.tile([P, P], BF16, tag="identB", name="identB")
        make_identity(nc, identB)
        eps_t = singles.tile([P, 1], FP32, tag="eps", name="eps")
        nc.vector.memset(eps_t, EPS)

        def col_load(vec_ap, n_ch, tag):
            """[n_ch*128] DRAM vector -> [128, n_ch] SBUF per-partition."""
            t = singles.tile([P, n_ch], FP32, tag=tag, name=tag)
            nc.sync.dma_start(out=t, in_=vec_ap.rearrange("(c p) -> p c", p=P))
            return t

        def bcast_load(vec_ap, tag):
            """[768] DRAM vector -> [128, 768] broadcast across partitions."""
            t = singles.tile([P, C], FP32, tag=tag, name=tag)
            src = bass.AP(
                tensor=vec_ap.tensor,
                offset=vec_ap.offset,
                ap=[[0, P], *vec_ap.ap],
            )
            nc.sync.dma_start(out=t, in_=src)
            return t

        # --- weights (fp8 pair layout) --------------------------------
        wqkv = [
            wpool.tile([P, 2, 3 * C], FP8, tag=f"wqkv{j}", name=f"wqkv{j}")
            for j in range(CP)
        ]
        wproj = [
            wpool.tile([P, 2, C], FP8, tag=f"wproj{j}", name=f"wproj{j}")
            for j in range(CP)
        ]
        wfc1 = [
            wpool.tile([P, 2, HID], FP8, tag=f"wfc1{j}", name=f"wfc1{j}")
            for j in range(CP)
        ]
        wfc2 = [
            wpool.tile([P, 2, C], FP8, tag=f"wfc2{j}", name=f"wfc2{j}")
            for j in range(HIDP)
        ]
        # --- load x first (LN1 is the critical path at kernel start) ---
        xt = [
            xpool.tile([P, C], FP32, tag=f"x{m}", name=f"x{m}")
            for m in range(TOK)
        ]
        # DMA issue order is Sync-queue execution order: x0-3 (LN1 can
        # start), ln1 gain/bias, qkv weights, rest of x, then the
        # late-needed weights/vectors (strided col_loads are
        # descriptor-expensive; keep them off the critical-path front).
        for m in range(4):
            nc.sync.dma_start(out=xt[m], in_=x[m * P : (m + 1) * P, :])
        g1c = col_load(ln1_g, CT, "g1c")
        b1c = col_load(ln1_b, CT, "b1c")
        for j in range(CP):
            nc.sync.dma_start(
                out=wqkv[j],
                in_=w_qkv[j * P : (j + 1) * P, :].rearrange("p (i c) -> p i c", i=2),
            )
        for m in range(4, TOK):
            nc.sync.dma_start(out=xt[m], in_=x[m * P : (m + 1) * P, :])
        bp_b = bcast_load(b_proj, "bp_b") if with_b_proj else None
        for j in range(CP):
            nc.sync.dma_start(
                out=wproj[j],
                in_=w_proj[j * P : (j + 1) * P, :].rearrange("p (i c) -> p i c", i=2),
            )
        g2c = col_load(ln2_g, CT, "g2c")
        b2c = col_load(ln2_b, CT, "b2c")
        for j in range(CP):
            nc.sync.dma_start(
                out=wfc1[j],
                in_=w_fc1[j * P : (j + 1) * P, :].rearrange("p (i c) -> p i c", i=2),
            )
        bf1c = col_load(b_fc1, HIDT, "bf1c")
        bf2_b = bcast_load(b_fc2, "bf2_b") if with_b_fc2 else None
        for j in range(HIDP):
            nc.sync.dma_start(
                out=wfc2[j],
                in_=w_fc2[j * P : (j + 1) * P, :].rearrange("p (i c) -> p i c", i=2),
            )

        def ln_normalize(src_tile):
            """token-major [128, 768] -> bf16 normalized (x-mu)*rstd."""
            st = stats.tile([P, 3, 6], FP32, tag="bnst", name="bnst")
            src3 = src_tile.rearrange("p (s d) -> p s d", s=3)
            for s in range(3):
                nc.vector.bn_stats(out=st[:, s, :], in_=src3[:, s, :])
            mv = stats.tile([P, 2], FP32, tag="bnmv", name="bnmv")
            nc.vector.bn_aggr(out=mv, in_=st)
            # rstd = exp(-0.5*ln(var+eps)); Ln+Exp share the exp table set,
            # so LN never forces an ACT table switch (Sqrt would).
            lnv = stats.tile([P, 1], FP32, tag="bnlnv", name="bnlnv")
            nc.scalar.activation(
                out=lnv, in_=mv[:, 1:2], func=AF.Ln, bias=eps_t, scale=1.0
            )
            rstd = stats.tile([P, 1], FP32, tag="bnrstd", name="bnrstd")
            nc.scalar.activation(out=rstd, in_=lnv, func=AF.Exp, scale=-0.5)
            # -mu*rstd so the normalize can run on ACT (free affine):
            # hn = Identity(x*rstd + (-mu*rstd))
            nmr = stats.tile([P, 1], FP32, tag="nmr", name="nmr")
            nc.vector.tensor_scalar(
                out=nmr, in0=mv[:, 0:1], scalar1=rstd, scalar2=-1.0,
                op0=ALU.mult, op1=ALU.mult,
            )
            hn = temps.tile([P, C], BF16, tag="hn", name="hn")
            nc.scalar.activation(
                out=hn, in_=src_tile, func=AF.Identity,
                scale=rstd, bias=nmr,
            )
            return hn

        def transpose_affine(hn, dstT_pairs, m, gcol, bcol, pspool, tag):
            """transpose bf16 token-major [128,768] into fp8 pair tiles'
            column m; g,b applied per-partition on ACT/DVE."""
            for c in range(CT):
                tp = pspool.tile([P, P], BF16, tag=tag, name=tag)
                nc.tensor.transpose(tp, hn[:, c * P : (c + 1) * P], identB)
                dst = dstT_pairs[c // 2][:, c % 2, m * P : (m + 1) * P]
                if c < CT // 2:
                    nc.scalar.activation(
                        out=dst, in_=tp, func=AF.Identity,
                        scale=gcol[:, c : c + 1], bias=bcol[:, c : c + 1],
                    )
                else:
                    nc.vector.tensor_scalar(
                        out=dst, in0=tp, scalar1=gcol[:, c : c + 1],
                        scalar2=bcol[:, c : c + 1], op0=ALU.mult, op1=ALU.add,
                    )

        with tc.tile_pool(name="hTpool", bufs=1) as hTpool:
            # --- LN1 + transpose -> hT pairs; fold b_proj into x -------
            hT = [
                hTpool.tile([P, 2, N], FP8, tag=f"hT{j}", name=f"hT{j}")
                for j in range(CP)
            ]
            with (
                tc.tile_pool(name="vxpool", bufs=1) as vxpool,
                tc.tile_pool(name="qkTpool", bufs=1) as qkTpool,
            ):
                # v in fp8 kc-pair layout for DoubleRow P@v: slot kc%2,
                # inner dim padded to 68 so the pair stride is 16-aligned
                vx = [
                    vxpool.tile(
                        [P, 2, H, HD + 4], FP8, tag=f"vx{kp}", name=f"vx{kp}"
                    )
                    for kp in range(TOK // 2)
                ]
                qkT = [
                    qkTpool.tile([P, N], BF16, tag=f"qkT{i}", name=f"qkT{i}")
                    for i in range(2 * CT)
                ]
                with (
                    tc.tile_pool(name="psA", bufs=4, space="PSUM") as psA,
                    tc.tile_pool(name="psQ", bufs=3, space="PSUM") as psQ,
                ):
                    def qkT_half(i, h):
                        """qkT tile i (i<6: q dims, else k dims), token
                        half h.  h=0 only needs token tiles 0-3, so it is
                        emitted mid-LN1 to fill the idle PE."""
                        col = i * P if i < CT else 3 * C // 2 + (i - CT) * P
                        ps = psQ.tile([P, 512], FP32, tag="qps", name="qps")
                        for j in range(CP):
                            nc.tensor.matmul(
                                ps,
                                lhsT=wqkv[j][:, :, col : col + P],
                                rhs=hT[j][:, :, h * 512 : (h + 1) * 512],
                                start=(j == 0),
                                stop=(j == CP - 1),
                                perf_mode=DR,
                            )
                        if h == 0:
                            # mid-LN1: DVE is the bottleneck, use ACT
                            nc.scalar.activation(
                                out=qkT[i][:, h * 512 : (h + 1) * 512],
                                in_=ps, func=AF.Identity,
                            )
                        else:
                            # post-LN1: ACT's queue gates the first
                            # scores; DVE is idle here
                            nc.vector.tensor_copy(
                                qkT[i][:, h * 512 : (h + 1) * 512], ps
                            )

                    for m in range(TOK):
                        hn = ln_normalize(xt[m])
                        transpose_affine(hn, hT, m, g1c, b1c, psA, "trA")
                        if with_b_proj:
                            nc.gpsimd.tensor_tensor(
                                out=xt[m], in0=xt[m], in1=bp_b, op=ALU.add
                            )
                        if m == 3:
                            for i in range(2 * CT):
                                qkT_half(i, 0)
                    for i in range(2 * CT):
                        qkT_half(i, 1)

                with tc.tile_pool(name="oTpool", bufs=1) as oTpool:
                    # --- per head-pair: qkT -> scores -> exp -> P@v ----
                    oT = [
                        oTpool.tile(
                            [P, 2, N], FP8, tag=f"oT{j}", name=f"oT{j}"
                        )
                        for j in range(CP)
                    ]
                    with (
                        tc.tile_pool(name="psS", bufs=3, space="PSUM") as psS,
                        tc.tile_pool(name="psO", bufs=2, space="PSUM") as psO,
                        tc.tile_pool(name="expp", bufs=1) as expp,
                        tc.tile_pool(name="attn_t", bufs=4) as attn_t,
                        tc.tile_pool(name="rsd", bufs=8, space="DRAM") as rsd,
                    ):
                        def make_v(m):
                            """v token tile via the score-psum ring."""
                            ps = psS.tile([P, 1024], FP32, tag="sps", name="vps")
                            for j in range(CP):
                                for n0, n1 in ((0, 512), (512, 768)):
                                    nc.tensor.matmul(
                                        ps[:, n0:n1],
                                        lhsT=hT[j][:, :, m * P : (m + 1) * P],
                                        rhs=wqkv[j][:, :, 2 * C + n0 : 2 * C + n1],
                                        start=(j == 0),
                                        stop=(j == CP - 1),
                                        perf_mode=DR,
                                    )
                            dst = vx[m // 2][:, m % 2, :, :]
                            nc.vector.memset(dst[:, :, HD : HD + 1], 1.0)
                            nc.vector.tensor_copy(
                                dst[:, :, 0:HD],
                                ps[:, 0:C].rearrange("p (h d) -> p h d", h=H),
                            )

                        # exp in fp8e4 (scores are in [-2.1, 2.1]; e4m3
                        # halves the quantization error vs e5m2).  Layout
                        # [P, kc%2, sub, qh*512+col] per kc-pair tile so
                        # one (kc,qh) psum tile exps into a contiguous
                        # [2,512] slice and the DoubleRow P@v reads
                        # [P, 2(kc), 512].
                        # double-buffered by head-pair parity so pair p's
                        # P@v chains (running during pair p+1's score
                        # phase) never race the new exps
                        expS2 = [
                            [
                                expp.tile(
                                    [P, 2, 2, N],
                                    FP8,
                                    tag=f"expS{par}_{kp}",
                                    name=f"expS{par}_{kp}",
                                )
                                for kp in range(TOK // 2)
                            ]
                            for par in range(2)
                        ]
                        EXP_SC = SCALE / (WS_QKV * WS_QKV)

                        def exp_emit(dst, sp, on_act):
                            if on_act:
                                nc.scalar.activation(
                                    out=dst, in_=sp, func=AF.Exp,
                                    scale=EXP_SC,
                                )
                            else:
                                nc.vector.tensor_scalar(
                                    out=dst.bitcast(U8), in0=sp,
                                    scalar1=SCH_A * EXP_SC,
                                    scalar2=SCH_B,
                                    op0=ALU.mult, op1=ALU.add,
                                )

                        def emit_scores(qt, kt, kc, par):
                            """Per q-half: one [128,1024] psum tile, two
                            row-group-paired score MMs (T0/T8 sharing the
                            tile, so the second MM carries no alloc wait
                            and the PE streams them concurrently).  exp
                            on ACT for qh0, Schraudolph-to-e4m3 bits on
                            DVE for qh1 (parallel engines)."""
                            for qh in range(2):
                                sp = psS.tile(
                                    [P, 1024], FP32, tag="sps", name="sps"
                                )
                                for sub in range(2):
                                    rows = slice(sub * HD, (sub + 1) * HD)
                                    nc.tensor.matmul(
                                        sp[:, sub * 512 : (sub + 1) * 512],
                                        lhsT=kt[rows, kc * P : (kc + 1) * P],
                                        rhs=qt[rows, qh * 512 : (qh + 1) * 512],
                                        start=True,
                                        stop=True,
                                    )
                                dst = expS2[par][kc // 2][
                                    :, kc % 2, :, qh * 512 : (qh + 1) * 512
                                ]
                                exp_emit(dst, sp, qh == 0)

                        def attn_chain_mms(hp, sub, chains, kps):
                            """kc-pair DoubleRow steps of the P@v
                            accumulation chains (both q-halves); batched
                            kps cut tiling-mode switches."""
                            head = 2 * hp + sub
                            for kp in kps:
                                for qh in range(2):
                                    nc.tensor.matmul(
                                        chains[qh][0 : HD + 1, :],
                                        lhsT=vx[kp][:, :, head, 0 : HD + 1],
                                        rhs=expS2[hp % 2][kp][
                                            :, :, sub, qh * 512 : (qh + 1) * 512
                                        ],
                                        start=(kp == 0),
                                        stop=(kp == TOK // 2 - 1),
                                        perf_mode=DR,
                                    )

                        def copy_out(chains):
                            """Numerators (+denominator row 64) to one
                            [65,1024] SBUF tile, freeing psO fast."""
                            oU = attn_t.tile(
                                [HD + 1, 1024], FP32, tag="oU", name="oU",
                                bufs=4,
                            )
                            for qh in range(2):
                                nc.vector.tensor_copy(
                                    oU[:, qh * 512 : (qh + 1) * 512],
                                    chains[qh][0 : HD + 1, :],
                                )
                            return oU

                        def bcast64(src_row):
                            """[1,1024] DRAM row -> [64,1024] stride-0."""
                            rbs = attn_t.tile(
                                [HD, 1024], FP32, tag="rbs", name="rbs",
                                bufs=2,
                            )
                            bsrc = bass.AP(
                                tensor=src_row.tensor,
                                offset=src_row.offset,
                                ap=[[0, HD], *src_row.ap[1:]],
                            )
                            nc.sync.dma_start(out=rbs, in_=bsrc)
                            return rbs

                        def norm_mult(oU, rbs, head, eng):
                            """oT[head] = oU * recip-denominator, one
                            [64,1024] op (both q-halves)."""
                            dst = oT[head // 4][
                                (head % 2) * HD : (head % 2 + 1) * HD,
                                (head // 2) % 2,
                                :,
                            ]
                            eng.tensor_tensor(
                                out=dst, in0=oU[0:HD, :], in1=rbs,
                                op=ALU.mult,
                            )

                        def denom_single(oU, hp, sub, eng):
                            """denominator row -> Ln -> Exp(-x) on ACT
                            (exp table set) -> DRAM -> [64,1024]
                            stride-0 broadcast -> normalize multiply."""
                            lnd = attn_t.tile(
                                [1, 1024], FP32, tag="lnd", name="lnd",
                                bufs=1,
                            )
                            nc.scalar.activation(
                                out=lnd, in_=oU[HD : HD + 1, :], func=AF.Ln,
                            )
                            rsf = attn_t.tile(
                                [1, 1024], FP32, tag="rsf", name="rsf",
                                bufs=1,
                            )
                            nc.scalar.activation(
                                out=rsf, in_=lnd, func=AF.Exp, scale=-1.0,
                            )
                            rd = rsd.tile([1, 1024], FP32, tag="rd", name="rd")
                            nc.sync.dma_start(out=rd, in_=rsf)
                            norm_mult(oU, bcast64(rd[0:1, :]), 2 * hp + sub, eng)

                        def denom_batch(oU0, oU1, hp):
                            """both subs' denominator rows gathered into
                            one [2,1024] tile (SBUF->SBUF DMA) -> one Ln
                            + one Exp -> DRAM -> two broadcasts."""
                            dn = attn_t.tile(
                                [2, 1024], FP32, tag="dn", name="dn", bufs=1
                            )
                            nc.sync.dma_start(
                                out=dn[0:1, :], in_=oU0[HD : HD + 1, :]
                            )
                            nc.sync.dma_start(
                                out=dn[1:2, :], in_=oU1[HD : HD + 1, :]
                            )
                            nc.scalar.activation(out=dn, in_=dn, func=AF.Ln)
                            rsf = attn_t.tile(
                                [2, 1024], FP32, tag="rsf2", name="rsf2",
                                bufs=1,
                            )
                            nc.scalar.activation(
                                out=rsf, in_=dn, func=AF.Exp, scale=-1.0,
                            )
                            rd = rsd.tile([2, 1024], FP32, tag="rd2", name="rd2")
                            nc.sync.dma_start(out=rd, in_=rsf)
                            return rd

                        def alloc_chains():
                            return {
                                qh: psO.tile(
                                    [P, 512], FP32, tag="ops", name="ops"
                                )
                                for qh in range(2)
                            }

                        # --- cross-pair pipeline: pair p's P@v chains,
                        # numerator copies and denominators all run
                        # inside pair p+1's score/exp phase (which is
                        # exp-paced, leaving PE and Sync slack).  Pair 0
                        # interleaves v production instead; the last
                        # pair starts its own sub0 chains early and
                        # finishes with a short DVE tail. -------------
                        HP = H // 2
                        for hp in range(HP):
                            qt, kt = qkT[hp], qkT[CT + hp]
                            lastp = hp == HP - 1
                            pv = {}
                            for kc in range(TOK):
                                emit_scores(qt, kt, kc, hp % 2)
                                if hp == 0:
                                    make_v(kc)
                                    continue
                                p = hp - 1
                                if kc == 0:
                                    pv["c0"] = alloc_chains()
                                    attn_chain_mms(p, 0, pv["c0"], (0, 1))
                                elif kc == 1:
                                    attn_chain_mms(p, 0, pv["c0"], (2, 3))
                                elif kc == 2:
                                    pv["oU0"] = copy_out(pv["c0"])
                                elif kc == 3:
                                    pv["c1"] = alloc_chains()
                                    attn_chain_mms(p, 1, pv["c1"], (0, 1))
                                elif kc == 4:
                                    attn_chain_mms(p, 1, pv["c1"], (2, 3))
                                elif kc == 5:
                                    pv["oU1"] = copy_out(pv["c1"])
                                    pv["rd"] = denom_batch(
                                        pv["oU0"], pv["oU1"], p
                                    )
                                    if lastp:
                                        pv["mine0"] = alloc_chains()
                                        attn_chain_mms(
                                            hp, 0, pv["mine0"], (0, 1)
                                        )
                                elif kc == 6:
                                    norm_mult(
                                        pv["oU0"], bcast64(pv["rd"][0:1, :]),
                                        2 * p, nc.gpsimd,
                                    )
                                    norm_mult(
                                        pv["oU1"], bcast64(pv["rd"][1:2, :]),
                                        2 * p + 1, nc.gpsimd,
                                    )
                                    if lastp:
                                        attn_chain_mms(
                                            hp, 0, pv["mine0"], (2,)
                                        )
                                elif kc == 7 and lastp:
                                    # sub1 chains ride a psS-ring tile
                                    # ([P,512] halves = one bank each) so
                                    # psO keeps holding sub0's
                                    spc = psS.tile(
                                        [P, 1024], FP32, tag="sps", name="c1s"
                                    )
                                    pv["c1m"] = {
                                        0: spc[:, 0:512],
                                        1: spc[:, 512:1024],
                                    }
                                    attn_chain_mms(hp, 1, pv["c1m"], (0, 1, 2))
                            if hp == 0 or not lastp:
                                continue
                            # --- last pair tail (short): only the kp3
                            # steps, numerator copies and the two
                            # denominator paths remain ----------------
                            attn_chain_mms(hp, 0, pv["mine0"], (3,))
                            oU0 = copy_out(pv["mine0"])
                            attn_chain_mms(hp, 1, pv["c1m"], (3,))
                            denom_single(oU0, hp, 0, nc.vector)
                            oU1 = copy_out(pv["c1m"])
                            denom_single(oU1, hp, 1, nc.vector)

                    # --- proj + residual + LN2, interleaved per token
                    # tile so LN2 (DVE) overlaps proj (PE) --------------
                    x1t = xt  # x tiles become x1 = x (+ b_proj) + attn
                    with tc.tile_pool(name="gTpool", bufs=1) as gTpool:
                      gT = [
                          gTpool.tile([P, 2, N], FP8, tag=f"gT{j}", name=f"gT{j}")
                          for j in range(HIDP)
                      ]
                      with tc.tile_pool(name="h2Tpool", bufs=1) as h2Tpool:
                        h2T = [
                            h2Tpool.tile([P, 2, N], FP8, tag=f"h2T{j}", name=f"h2T{j}")
                            for j in range(CP)
                        ]
                        with (
                            tc.tile_pool(name="psP", bufs=3, space="PSUM") as psP,
                            tc.tile_pool(name="psT2", bufs=2, space="PSUM") as psT2,
                        ):
                            for m in range(TOK):
                                ps = psP.tile([P, C], FP32, tag="pps", name="pps")
                                for j in range(CP):
                                    for n0, n1 in ((0, 512), (512, 768)):
                                        nc.tensor.matmul(
                                            ps[:, n0:n1],
                                            lhsT=oT[j][:, :, m * P : (m + 1) * P],
                                            rhs=wproj[j][:, :, n0:n1],
                                            start=(j == 0),
                                            stop=(j == CP - 1),
                                            perf_mode=DR,
                                        )
                                nc.vector.scalar_tensor_tensor(
                                    out=xt[m], in0=ps,
                                    scalar=1.0 / (WS_QKV * WS_PROJ),
                                    in1=xt[m],
                                    op0=ALU.mult, op1=ALU.add,
                                )
                                hn = ln_normalize(x1t[m])
                                transpose_affine(hn, h2T, m, g2c, b2c, psT2, "trB")
                                if with_b_fc2:
                                    nc.gpsimd.tensor_tensor(
                                        out=x1t[m], in0=x1t[m], in1=bf2_b, op=ALU.add
                                    )

                        # --- fc1 + gelu -> gT pairs ------------------------
                        # h outer: the h=0 half only needs LN2 of
                        # token tiles 0-3, so it starts ~8us earlier
                        with tc.tile_pool(name="psU", bufs=3, space="PSUM") as psU:
                            # mh pairs share one [128,1024] psum tile so
                            # gelu runs at half the per-call overhead
                            # (bias must be per-partition-constant, so
                            # pairing needs b_fc1 == 0; else fall back);
                            # h-outer keeps the h=0 half starting early
                            # (needs LN2 of token tiles 0-3 only).
                            for h in range(2):
                                for mhp in range(HIDP):
                                    ps = psU.tile([P, 1024], FP32, tag="ups", name="ups")
                                    for i in range(2):
                                        mh = 2 * mhp + i
                                        for j in range(CP):
                                            nc.tensor.matmul(
                                                ps[:, i * 512 : (i + 1) * 512],
                                                lhsT=wfc1[j][:, :, mh * P : (mh + 1) * P],
                                                rhs=h2T[j][:, :, h * 512 : (h + 1) * 512],
                                                start=(j == 0),
                                                stop=(j == CP - 1),
                                                perf_mode=DR,
                                            )
                                    if with_b_fc1:
                                        for i in range(2):
                                            nc.scalar.activation(
                                                out=gT[mhp][:, i, h * 512 : (h + 1) * 512],
                                                in_=ps[:, i * 512 : (i + 1) * 512],
                                                func=AF.Gelu,
                                                bias=bf1c[:, 2 * mhp + i : 2 * mhp + i + 1],
                                                scale=1.0 / WS_FC1,
                                            )
                                    else:
                                        nc.scalar.activation(
                                            out=gT[mhp][:, :, h * 512 : (h + 1) * 512],
                                            in_=ps, func=AF.Gelu,
                                            scale=1.0 / WS_FC1,
                                        )

                      # --- fc2 token-major: out[m] = x1[m] + gT.T @ wfc2 -
                      with tc.tile_pool(name="psY", bufs=2, space="PSUM") as psY:
                        for m in range(TOK):
                            ps = psY.tile([P, C], FP32, tag="yps", name="yps")
                            for j in range(HIDP):
                                for n0, n1 in ((0, 512), (512, 768)):
                                    nc.tensor.matmul(
                                        ps[:, n0:n1],
                                        lhsT=gT[j][:, :, m * P : (m + 1) * P],
                                        rhs=wfc2[j][:, :, n0:n1],
                                        start=(j == 0),
                                        stop=(j == HIDP - 1),
                                        perf_mode=DR,
                                    )
                            nc.vector.scalar_tensor_tensor(
                                out=x1t[m], in0=ps, scalar=1.0 / WS_FC2,
                                in1=x1t[m], op0=ALU.mult, op1=ALU.add,
                            )
                            nc.sync.dma_start(
                                out=out[m * P : (m + 1) * P, :], in_=x1t[m]
                            )

    ctx_lp.__exit__(None, None, None)
    return out


# ---- wait splitting (walrus allows 1 sync wait/instruction) ----

"""Post-pass: this container's walrus rejects >1 sync wait per instruction.

Tile's sem-assignment freely attaches several waits to one instruction.
Peel all but the last wait onto freshly inserted NoOp instructions on the
same engine, placed immediately before the instruction in its block.
"""


def split_multi_waits(nc, max_waits: int = 1) -> int:
    n_split = 0
    for f in nc.m.functions:
        for bb in f.blocks:
            insts = list(bb.instructions)
            out = []
            for inst in insts:
                si = inst.sync_info
                waits = list(si.on_wait) if si is not None else []
                if len(waits) > max_waits:
                    n_split += 1
                    peel = waits[:-max_waits]
                    si.on_wait = waits[-max_waits:]
                    for i in range(0, len(peel), max_waits):
                        nop = mybir.InstNoOp(
                            name=f"I-waitfix-{n_split}-{i}",
                            engine=inst.engine,
                            ins=[],
                            outs=[],
                            sync_info=mybir.SyncInfo(
                                on_wait=peel[i : i + max_waits], on_update=[]
                            ),
                        )
                        nc.register_instruction(nop)
                        out.append(nop)
                out.append(inst)
            if len(out) != len(insts):
                bb.instructions[:] = out
    return n_split


# ----------------------------------------------------------------------
# SPMD entry point: full inputs in, full outputs out (8-way batch-parallel)
# ----------------------------------------------------------------------
import numpy as _np
import ml_dtypes as _mld

_N_CORES = 8
_FP32_KEYS = ["ln1_g", "ln1_b", "b_proj", "ln2_g", "ln2_b", "b_fc1", "b_fc2"]


def _pair_fp8(w):
    """[K, M] fp32 -> [K/2 * 128?, ...] pair layout: out[j*128+p, i*M+c] =
    w[256j+128i+p, c], cast to fp8e4m3."""
    K, M = w.shape
    JP = K // 256
    w8 = w.astype(_mld.float8_e4m3fn)
    w4 = w8.reshape(JP, 2, P, M).transpose(0, 2, 1, 3)  # [j, p, i, c]
    return _np.ascontiguousarray(w4).reshape(JP * P, 2 * M)


def _prep_weights(inputs):
    w = {}
    for k in _FP32_KEYS:
        w[k] = _np.ascontiguousarray(_np.asarray(inputs[k], dtype=_np.float32))
    w["wqkv_p"] = _pair_fp8(_np.asarray(inputs["w_qkv"], dtype=_np.float32) * WS_QKV)
    w["wproj_p"] = _pair_fp8(_np.asarray(inputs["w_proj"], dtype=_np.float32) * WS_PROJ)
    w["wfc1_p"] = _pair_fp8(_np.asarray(inputs["w_fc1"], dtype=_np.float32) * WS_FC1)
    w["wfc2_p"] = _pair_fp8(_np.asarray(inputs["w_fc2"], dtype=_np.float32) * WS_FC2)
    return w


def _build_program(weights):
    import concourse.tile as tile

    nc = bass.Bass("TRN2", target_bir_lowering=False, debug=False,
                   num_devices=_N_CORES)
    with tile.TileContext(nc) as tc:
        build(
            nc, tc,
            with_b_proj=bool(_np.any(weights["b_proj"])),
            with_b_fc2=bool(_np.any(weights["b_fc2"])),
            with_b_fc1=bool(_np.any(weights["b_fc1"])),
        )
    split_multi_waits(nc)
    return nc


def kernel(**inputs):
    from concourse.bass_utils import run_bass_kernel_spmd

    x = _np.ascontiguousarray(_np.asarray(inputs["x"], dtype=_np.float32))
    assert x.shape == (8, N, C), x.shape
    weights = _prep_weights(inputs)
    nc = _build_program(weights)
    in_maps = [{"x": x[b], **weights} for b in range(_N_CORES)]
    res = run_bass_kernel_spmd(nc, in_maps, list(range(_N_CORES)))
    out = _np.stack([res.results[b]["out"] for b in range(_N_CORES)])
    return out.astype(_np.float32)



# revision 32
# speedup vs baseline: 1.0766x; 1.0128x over previous
"""Self-contained Trainium2 kernel for nn_Block (dense transformer block),
8-way batch-parallel across NeuronCores.  V2: fp8e4m3 DoubleRow matmuls
for qkv/qkT/proj/fc1/fc2, token-major fc2 (no output transposes),
softmax denominators via DVE reciprocal (off the ACT critical path),
exp in [128,1024] tiles.

Per-core program: one transformer block over one batch element
x[1024, 768] -> out[1024, 768].

Layouts: token-major = tokens on partitions; feature-major = channels on
partitions.  LN runs token-major (free-dim stats via bn_stats), then
PE-transposes into feature-major with the LN gain/bias fused into the
PSUM evacuation (per-partition scale/bias APs), quantizing to fp8 in
"pair" layout [128, 2, N] where slot i holds channel rows 256j+128i+p
(DoubleRow contraction over K=256 per matmul pass).

Weights arrive in DRAM pre-cast to fp8e4m3 in matching pair layout,
pre-scaled x16 (x32 for fc2) so U(-1/sqrt(fan_in),..) values clear
e4m3's subnormal cutoff; descales fold into exp's scale, gelu's input
scale, and scalar_tensor_tensor evacuations.  Scores S^T are k-major
bf16 (two heads row-group-paired); exp on ACT over [128,1024] PSUM
tiles, output fp8e5m2 in kc-pair layout; P@v runs DoubleRow with fp8 v
(token-major, ones column per head emitting softmax denominators in
row 64).  Denominator pairs gather via DRAM into [2,512] -> Ln+Exp(-x)
on ACT (same table set as exp; DVE reciprocal is 3.2us/call and the
recip table set would thrash) -> DRAM -> [64,512] stride-0 broadcast
-> DVE multiply.  Accumulation stays fp32 in PSUM.  fc2 runs
token-major (lhsT=gT chunks): residual add + store, no transposes.

Known limit: the ACT-paced attention leaves the PE at ~60% duty so the
HAM clock gate keeps it at 1.2 GHz there (score MMs ~427ns = 512/1.2);
dense MLP streams run warm at 2.4 GHz (DR MMs 229ns).  Warm-up bursts
+ tiny-MM heartbeats do NOT hold K=8/8 (micro-idles re-throttle) -
tried and reverted.  NOTE: exec time is bimodal (~315-321us fast mode,
~377us occasional slow mode, code-independent) - judge changes on 2-3
runs, never one.
"""

import concourse.bass as bass
import concourse.mybir as mybir
from concourse.masks import make_identity

AF = mybir.ActivationFunctionType
ALU = mybir.AluOpType
FP32 = mybir.dt.float32
BF16 = mybir.dt.bfloat16
FP8 = mybir.dt.float8e4
FP8E5 = mybir.dt.float8e5
U8 = mybir.dt.uint8
DR = mybir.MatmulPerfMode.DoubleRow
# Schraudolph constant: e4m3 bits of exp(x) = round(x*8/ln2 + 56)
SCH_A = 8.0 / 0.6931471805599453
SCH_B = 56.0

N, C, H, HD, HID = 1024, 768, 12, 64, 4 * 768
P = 128
TOK = N // P  # 8 token chunks
CT = C // P  # 6 channel chunks
CP = CT // 2  # 3 channel pairs (K=256 DoubleRow passes)
HIDT = HID // P  # 24 hidden chunks
HIDP = HIDT // 2  # 12 hidden pairs
EPS = 1e-5
SCALE = HD ** (-0.5)
# fp8 weight pre-scaling: U(-1/sqrt(fan_in), ..) weights sit below
# e4m3's normal range (2^-6); scale up before the cast, descale via
# existing free op parameters (exp scale, gelu input scale,
# scalar_tensor_tensor evacuations).
WS_QKV = 16.0
WS_PROJ = 16.0
WS_FC1 = 16.0
WS_FC2 = 32.0


def build(nc: bass.Bass, tc, with_b_proj=True, with_b_fc2=True, with_b_fc1=True):
    ctx_lp = nc.allow_low_precision(
        reason="fp8 DoubleRow matmuls, fp32 accum; validated vs fp32 reference"
    )
    ctx_lp.__enter__()
    x = nc.dram_tensor("x", [N, C], FP32, kind="ExternalInput").ap()
    ln1_g = nc.dram_tensor("ln1_g", [C], FP32, kind="ExternalInput").ap()
    ln1_b = nc.dram_tensor("ln1_b", [C], FP32, kind="ExternalInput").ap()
    # pair-layout fp8 weights (host-prepped):
    #   wqkv_p[j*128+p, i*2304+c] = w_qkv[256j+128i+p, c]
    w_qkv = nc.dram_tensor("wqkv_p", [CP * P, 2 * 3 * C], FP8, kind="ExternalInput").ap()
    w_proj = nc.dram_tensor("wproj_p", [CP * P, 2 * C], FP8, kind="ExternalInput").ap()
    b_proj = nc.dram_tensor("b_proj", [C], FP32, kind="ExternalInput").ap()
    ln2_g = nc.dram_tensor("ln2_g", [C], FP32, kind="ExternalInput").ap()
    ln2_b = nc.dram_tensor("ln2_b", [C], FP32, kind="ExternalInput").ap()
    w_fc1 = nc.dram_tensor("wfc1_p", [CP * P, 2 * HID], FP8, kind="ExternalInput").ap()
    b_fc1 = nc.dram_tensor("b_fc1", [HID], FP32, kind="ExternalInput").ap()
    w_fc2 = nc.dram_tensor("wfc2_p", [HIDP * P, 2 * C], FP8, kind="ExternalInput").ap()
    b_fc2 = nc.dram_tensor("b_fc2", [C], FP32, kind="ExternalInput").ap()
    out = nc.dram_tensor("out", [N, C], FP32, kind="ExternalOutput").ap()

    with (
        tc.tile_pool(name="singles", bufs=1) as singles,
        tc.tile_pool(name="xpool", bufs=1) as xpool,
        tc.tile_pool(name="temps", bufs=3) as temps,
        tc.tile_pool(name="stats", bufs=4) as stats,
        tc.tile_pool(name="wpool", bufs=1) as wpool,
    ):
        # --- constants -------------------------------------------------
        identB = singles.tile([P, P], BF16, tag="identB", name="identB")
        make_identity(nc, identB)
        eps_t = singles.tile([P, 1], FP32, tag="eps", name="eps")
        nc.vector.memset(eps_t, EPS)
        onesb = singles.tile([1, HD], FP32, tag="onesb", name="onesb")
        nc.vector.memset(onesb, 1.0)

        def col_load(vec_ap, n_ch, tag):
            """[n_ch*128] DRAM vector -> [128, n_ch] SBUF per-partition."""
            t = singles.tile([P, n_ch], FP32, tag=tag, name=tag)
            nc.sync.dma_start(out=t, in_=vec_ap.rearrange("(c p) -> p c", p=P))
            return t

        def bcast_load(vec_ap, tag):
            """[768] DRAM vector -> [128, 768] broadcast across partitions."""
            t = singles.tile([P, C], FP32, tag=tag, name=tag)
            src = bass.AP(
                tensor=vec_ap.tensor,
                offset=vec_ap.offset,
                ap=[[0, P], *vec_ap.ap],
            )
            nc.sync.dma_start(out=t, in_=src)
            return t

        # --- weights (fp8 pair layout) --------------------------------
        wqkv = [
            wpool.tile([P, 2, 3 * C], FP8, tag=f"wqkv{j}", name=f"wqkv{j}")
            for j in range(CP)
        ]
        wproj = [
            wpool.tile([P, 2, C], FP8, tag=f"wproj{j}", name=f"wproj{j}")
            for j in range(CP)
        ]
        wfc1 = [
            wpool.tile([P, 2, HID], FP8, tag=f"wfc1{j}", name=f"wfc1{j}")
            for j in range(CP)
        ]
        wfc2 = [
            wpool.tile([P, 2, C], FP8, tag=f"wfc2{j}", name=f"wfc2{j}")
            for j in range(HIDP)
        ]
        # --- load x first (LN1 is the critical path at kernel start) ---
        xt = [
            xpool.tile([P, C], FP32, tag=f"x{m}", name=f"x{m}")
            for m in range(TOK)
        ]
        # DMA issue order is Sync-queue execution order: x0-3 (LN1 can
        # start), ln1 gain/bias, qkv weights, rest of x, then the
        # late-needed weights/vectors (strided col_loads are
        # descriptor-expensive; keep them off the critical-path front).
        for m in range(4):
            nc.sync.dma_start(out=xt[m], in_=x[m * P : (m + 1) * P, :])
        g1c = col_load(ln1_g, CT, "g1c")
        b1c = col_load(ln1_b, CT, "b1c")
        for j in range(CP):
            nc.sync.dma_start(
                out=wqkv[j],
                in_=w_qkv[j * P : (j + 1) * P, :].rearrange("p (i c) -> p i c", i=2),
            )
        for m in range(4, TOK):
            nc.sync.dma_start(out=xt[m], in_=x[m * P : (m + 1) * P, :])
        bp_b = bcast_load(b_proj, "bp_b") if with_b_proj else None
        for j in range(CP):
            nc.sync.dma_start(
                out=wproj[j],
                in_=w_proj[j * P : (j + 1) * P, :].rearrange("p (i c) -> p i c", i=2),
            )
        g2c = col_load(ln2_g, CT, "g2c")
        b2c = col_load(ln2_b, CT, "b2c")
        for j in range(CP):
            nc.sync.dma_start(
                out=wfc1[j],
                in_=w_fc1[j * P : (j + 1) * P, :].rearrange("p (i c) -> p i c", i=2),
            )
        bf1c = col_load(b_fc1, HIDT, "bf1c")
        bf2_b = bcast_load(b_fc2, "bf2_b") if with_b_fc2 else None
        for j in range(HIDP):
            nc.sync.dma_start(
                out=wfc2[j],
                in_=w_fc2[j * P : (j + 1) * P, :].rearrange("p (i c) -> p i c", i=2),
            )

        def ln_normalize(src_tile):
            """token-major [128, 768] -> bf16 normalized (x-mu)*rstd."""
            st = stats.tile([P, 3, 6], FP32, tag="bnst", name="bnst")
            src3 = src_tile.rearrange("p (s d) -> p s d", s=3)
            for s in range(3):
                nc.vector.bn_stats(out=st[:, s, :], in_=src3[:, s, :])
            mv = stats.tile([P, 2], FP32, tag="bnmv", name="bnmv")
            nc.vector.bn_aggr(out=mv, in_=st)
            # rstd = exp(-0.5*ln(var+eps)); Ln+Exp share the exp table set,
            # so LN never forces an ACT table switch (Sqrt would).
            lnv = stats.tile([P, 1], FP32, tag="bnlnv", name="bnlnv")
            nc.scalar.activation(
                out=lnv, in_=mv[:, 1:2], func=AF.Ln, bias=eps_t, scale=1.0
            )
            rstd = stats.tile([P, 1], FP32, tag="bnrstd", name="bnrstd")
            nc.scalar.activation(out=rstd, in_=lnv, func=AF.Exp, scale=-0.5)
            # -mu*rstd so the normalize can run on ACT (free affine):
            # hn = Identity(x*rstd + (-mu*rstd))
            nmr = stats.tile([P, 1], FP32, tag="nmr", name="nmr")
            nc.vector.tensor_scalar(
                out=nmr, in0=mv[:, 0:1], scalar1=rstd, scalar2=-1.0,
                op0=ALU.mult, op1=ALU.mult,
            )
            hn = temps.tile([P, C], BF16, tag="hn", name="hn")
            nc.scalar.activation(
                out=hn, in_=src_tile, func=AF.Identity,
                scale=rstd, bias=nmr,
            )
            return hn

        def transpose_affine(hn, dstT_pairs, m, gcol, bcol, pspool, tag):
            """transpose bf16 token-major [128,768] into fp8 pair tiles'
            column m; g,b applied per-partition on ACT/DVE."""
            for c in range(CT):
                tp = pspool.tile([P, P], BF16, tag=tag, name=tag)
                nc.tensor.transpose(tp, hn[:, c * P : (c + 1) * P], identB)
                dst = dstT_pairs[c // 2][:, c % 2, m * P : (m + 1) * P]
                if c < CT // 2:
                    nc.scalar.activation(
                        out=dst, in_=tp, func=AF.Identity,
                        scale=gcol[:, c : c + 1], bias=bcol[:, c : c + 1],
                    )
                else:
                    nc.vector.tensor_scalar(
                        out=dst, in0=tp, scalar1=gcol[:, c : c + 1],
                        scalar2=bcol[:, c : c + 1], op0=ALU.mult, op1=ALU.add,
                    )

        with tc.tile_pool(name="hTpool", bufs=1) as hTpool:
            # --- LN1 + transpose -> hT pairs; fold b_proj into x -------
            hT = [
                hTpool.tile([P, 2, N], FP8, tag=f"hT{j}", name=f"hT{j}")
                for j in range(CP)
            ]
            with (
                tc.tile_pool(name="vxpool", bufs=1) as vxpool,
                tc.tile_pool(name="qkTpool", bufs=1) as qkTpool,
            ):
                # v in fp8 kc-pair layout for DoubleRow P@v: slot kc%2,
                # inner dim padded to 68 so the pair stride is 16-aligned
                vx = [
                    vxpool.tile(
                        [P, 2, H, HD + 4], FP8, tag=f"vx{kp}", name=f"vx{kp}"
                    )
                    for kp in range(TOK // 2)
                ]
                qkT = [
                    qkTpool.tile([P, N], BF16, tag=f"qkT{i}", name=f"qkT{i}")
                    for i in range(2 * CT)
                ]
                with (
                    tc.tile_pool(name="psA", bufs=4, space="PSUM") as psA,
                    tc.tile_pool(name="psQ", bufs=3, space="PSUM") as psQ,
                ):
                    def qkT_half(i, h):
                        """qkT tile i (i<6: q dims, else k dims), token
                        half h.  h=0 only needs token tiles 0-3, so it is
                        emitted mid-LN1 to fill the idle PE."""
                        col = i * P if i < CT else 3 * C // 2 + (i - CT) * P
                        ps = psQ.tile([P, 512], FP32, tag="qps", name="qps")
                        for j in range(CP):
                            nc.tensor.matmul(
                                ps,
                                lhsT=wqkv[j][:, :, col : col + P],
                                rhs=hT[j][:, :, h * 512 : (h + 1) * 512],
                                start=(j == 0),
                                stop=(j == CP - 1),
                                perf_mode=DR,
                            )
                        if h == 0:
                            # mid-LN1: DVE is the bottleneck, use ACT
                            nc.scalar.activation(
                                out=qkT[i][:, h * 512 : (h + 1) * 512],
                                in_=ps, func=AF.Identity,
                            )
                        else:
                            # post-LN1: ACT's queue gates the first
                            # scores; DVE is idle here
                            nc.vector.tensor_copy(
                                qkT[i][:, h * 512 : (h + 1) * 512], ps
                            )

                    for m in range(TOK):
                        hn = ln_normalize(xt[m])
                        transpose_affine(hn, hT, m, g1c, b1c, psA, "trA")
                        if with_b_proj:
                            nc.gpsimd.tensor_tensor(
                                out=xt[m], in0=xt[m], in1=bp_b, op=ALU.add
                            )
                        if m == 3:
                            for i in range(2 * CT):
                                qkT_half(i, 0)
                    for i in range(2 * CT):
                        qkT_half(i, 1)

                with tc.tile_pool(name="oTpool", bufs=1) as oTpool:
                    # --- per head-pair: qkT -> scores -> exp -> P@v ----
                    oT = [
                        oTpool.tile(
                            [P, 2, N], FP8, tag=f"oT{j}", name=f"oT{j}"
                        )
                        for j in range(CP)
                    ]
                    with (
                        tc.tile_pool(name="psS", bufs=3, space="PSUM") as psS,
                        tc.tile_pool(name="psO", bufs=2, space="PSUM") as psO,
                        tc.tile_pool(name="expp", bufs=1) as expp,
                        tc.tile_pool(name="attn_t", bufs=4) as attn_t,
                        tc.tile_pool(name="rsd", bufs=8, space="DRAM") as rsd,
                    ):
                        def make_v(m):
                            """v token tile via the score-psum ring."""
                            ps = psS.tile([P, 1024], FP32, tag="sps", name="vps")
                            for j in range(CP):
                                for n0, n1 in ((0, 512), (512, 768)):
                                    nc.tensor.matmul(
                                        ps[:, n0:n1],
                                        lhsT=hT[j][:, :, m * P : (m + 1) * P],
                                        rhs=wqkv[j][:, :, 2 * C + n0 : 2 * C + n1],
                                        start=(j == 0),
                                        stop=(j == CP - 1),
                                        perf_mode=DR,
                                    )
                            dst = vx[m // 2][:, m % 2, :, :]
                            nc.vector.memset(dst[:, :, HD : HD + 1], 1.0)
                            nc.vector.tensor_copy(
                                dst[:, :, 0:HD],
                                ps[:, 0:C].rearrange("p (h d) -> p h d", h=H),
                            )

                        # exp in fp8e4 (scores are in [-2.1, 2.1]; e4m3
                        # halves the quantization error vs e5m2).  Layout
                        # [P, kc%2, sub, qh*512+col] per kc-pair tile so
                        # one (kc,qh) psum tile exps into a contiguous
                        # [2,512] slice and the DoubleRow P@v reads
                        # [P, 2(kc), 512].
                        # double-buffered by head-pair parity so pair p's
                        # P@v chains (running during pair p+1's score
                        # phase) never race the new exps
                        expS2 = [
                            [
                                expp.tile(
                                    [P, 2, 2, N],
                                    FP8,
                                    tag=f"expS{par}_{kp}",
                                    name=f"expS{par}_{kp}",
                                )
                                for kp in range(TOK // 2)
                            ]
                            for par in range(2)
                        ]
                        EXP_SC = SCALE / (WS_QKV * WS_QKV)

                        def exp_emit(dst, sp, on_act):
                            if on_act:
                                nc.scalar.activation(
                                    out=dst, in_=sp, func=AF.Exp,
                                    scale=EXP_SC,
                                )
                            else:
                                nc.vector.tensor_scalar(
                                    out=dst.bitcast(U8), in0=sp,
                                    scalar1=SCH_A * EXP_SC,
                                    scalar2=SCH_B,
                                    op0=ALU.mult, op1=ALU.add,
                                )

                        def emit_scores(qt, kt, kc, par):
                            """Per q-half: one [128,1024] psum tile, two
                            row-group-paired score MMs (T0/T8 sharing the
                            tile, so the second MM carries no alloc wait
                            and the PE streams them concurrently).  exp
                            on ACT for qh0, Schraudolph-to-e4m3 bits on
                            DVE for qh1 (parallel engines)."""
                            for qh in range(2):
                                sp = psS.tile(
                                    [P, 1024], FP32, tag="sps", name="sps"
                                )
                                for sub in range(2):
                                    rows = slice(sub * HD, (sub + 1) * HD)
                                    nc.tensor.matmul(
                                        sp[:, sub * 512 : (sub + 1) * 512],
                                        lhsT=kt[rows, kc * P : (kc + 1) * P],
                                        rhs=qt[rows, qh * 512 : (qh + 1) * 512],
                                        start=True,
                                        stop=True,
                                    )
                                dst = expS2[par][kc // 2][
                                    :, kc % 2, :, qh * 512 : (qh + 1) * 512
                                ]
                                exp_emit(dst, sp, qh == 0)

                        def attn_chain_mms(hp, sub, chains, kps):
                            """kc-pair DoubleRow steps of the P@v
                            accumulation chains (both q-halves); batched
                            kps cut tiling-mode switches."""
                            head = 2 * hp + sub
                            for kp in kps:
                                for qh in range(2):
                                    nc.tensor.matmul(
                                        chains[qh][0 : HD + 1, :],
                                        lhsT=vx[kp][:, :, head, 0 : HD + 1],
                                        rhs=expS2[hp % 2][kp][
                                            :, :, sub, qh * 512 : (qh + 1) * 512
                                        ],
                                        start=(kp == 0),
                                        stop=(kp == TOK // 2 - 1),
                                        perf_mode=DR,
                                    )

                        def copy_out(chains):
                            """Numerators (+denominator row 64) to one
                            [65,1024] SBUF tile, freeing psO fast."""
                            oU = attn_t.tile(
                                [HD + 1, 1024], FP32, tag="oU", name="oU",
                                bufs=4,
                            )
                            for qh in range(2):
                                nc.vector.tensor_copy(
                                    oU[:, qh * 512 : (qh + 1) * 512],
                                    chains[qh][0 : HD + 1, :],
                                )
                            return oU

                        def bcast64(src_row):
                            """[1,1024] DRAM row -> [64,1024] stride-0."""
                            rbs = attn_t.tile(
                                [HD, 1024], FP32, tag="rbs", name="rbs",
                                bufs=3,
                            )
                            bsrc = bass.AP(
                                tensor=src_row.tensor,
                                offset=src_row.offset,
                                ap=[[0, HD], *src_row.ap[1:]],
                            )
                            nc.sync.dma_start(out=rbs, in_=bsrc)
                            return rbs

                        def norm_mult(oU, rbs, head, eng):
                            """oT[head] = oU * recip-denominator, one
                            [64,1024] op (both q-halves)."""
                            dst = oT[head // 4][
                                (head % 2) * HD : (head % 2 + 1) * HD,
                                (head // 2) % 2,
                                :,
                            ]
                            eng.tensor_tensor(
                                out=dst, in0=oU[0:HD, :], in1=rbs,
                                op=ALU.mult,
                            )

                        def denom_last(oU, hp, sub):
                            """tail path with no DMA hops: Ln -> Exp(-x)
                            on ACT, then a K=1 ones-matmul broadcasts
                            the [1,1024] reciprocal row across 64 PSUM
                            partitions, and DVE multiplies from PSUM."""
                            head = 2 * hp + sub
                            lnd = attn_t.tile(
                                [1, 1024], FP32, tag="lnd", name="lnd",
                                bufs=1,
                            )
                            nc.scalar.activation(
                                out=lnd, in_=oU[HD : HD + 1, :], func=AF.Ln,
                            )
                            rsf = attn_t.tile(
                                [1, 1024], FP32, tag="rsf", name="rsf",
                                bufs=1,
                            )
                            nc.scalar.activation(
                                out=rsf, in_=lnd, func=AF.Exp, scale=-1.0,
                            )
                            for qh in range(2):
                                pb = psO.tile(
                                    [P, 512], FP32, tag="ops", name="pb"
                                )
                                nc.tensor.matmul(
                                    pb[0:HD, :],
                                    lhsT=onesb,
                                    rhs=rsf[0:1, qh * 512 : (qh + 1) * 512],
                                    start=True,
                                    stop=True,
                                )
                                dst = oT[head // 4][
                                    (head % 2) * HD : (head % 2 + 1) * HD,
                                    (head // 2) % 2,
                                    qh * 512 : (qh + 1) * 512,
                                ]
                                nc.vector.tensor_tensor(
                                    out=dst,
                                    in0=oU[0:HD, qh * 512 : (qh + 1) * 512],
                                    in1=pb[0:HD, :],
                                    op=ALU.mult,
                                )

                        def denom_batch(oU0, oU1, hp):
                            """both subs' denominator rows gathered into
                            one [2,1024] tile (SBUF->SBUF DMA) -> one Ln
                            + one Exp -> DRAM -> two broadcasts."""
                            dn = attn_t.tile(
                                [2, 1024], FP32, tag="dn", name="dn", bufs=1
                            )
                            nc.sync.dma_start(
                                out=dn[0:1, :], in_=oU0[HD : HD + 1, :]
                            )
                            nc.sync.dma_start(
                                out=dn[1:2, :], in_=oU1[HD : HD + 1, :]
                            )
                            nc.scalar.activation(out=dn, in_=dn, func=AF.Ln)
                            rsf = attn_t.tile(
                                [2, 1024], FP32, tag="rsf2", name="rsf2",
                                bufs=1,
                            )
                            nc.scalar.activation(
                                out=rsf, in_=dn, func=AF.Exp, scale=-1.0,
                            )
                            rd = rsd.tile([2, 1024], FP32, tag="rd2", name="rd2")
                            nc.sync.dma_start(out=rd, in_=rsf)
                            return rd

                        def alloc_chains():
                            return {
                                qh: psO.tile(
                                    [P, 512], FP32, tag="ops", name="ops"
                                )
                                for qh in range(2)
                            }

                        # --- cross-pair pipeline: pair p's P@v chains,
                        # numerator copies and denominators all run
                        # inside pair p+1's score/exp phase (which is
                        # exp-paced, leaving PE and Sync slack).  Pair 0
                        # interleaves v production instead; the last
                        # pair starts its own sub0 chains early and
                        # finishes with a short DVE tail. -------------
                        HP = H // 2
                        for hp in range(HP):
                            qt, kt = qkT[hp], qkT[CT + hp]
                            lastp = hp == HP - 1
                            pv = {}
                            for kc in range(TOK):
                                emit_scores(qt, kt, kc, hp % 2)
                                if hp == 0:
                                    make_v(kc)
                                    continue
                                p = hp - 1
                                if kc == 0:
                                    pv["c0"] = alloc_chains()
                                    attn_chain_mms(p, 0, pv["c0"], (0, 1))
                                elif kc == 1:
                                    attn_chain_mms(p, 0, pv["c0"], (2, 3))
                                elif kc == 2:
                                    pv["oU0"] = copy_out(pv["c0"])
                                elif kc == 3:
                                    pv["c1"] = alloc_chains()
                                    attn_chain_mms(p, 1, pv["c1"], (0, 1))
                                elif kc == 4:
                                    attn_chain_mms(p, 1, pv["c1"], (2, 3))
                                elif kc == 5:
                                    pv["oU1"] = copy_out(pv["c1"])
                                    pv["rd"] = denom_batch(
                                        pv["oU0"], pv["oU1"], p
                                    )
                                    if lastp:
                                        pv["mine0"] = alloc_chains()
                                        attn_chain_mms(
                                            hp, 0, pv["mine0"], (0, 1)
                                        )
                                elif kc == 6:
                                    norm_mult(
                                        pv["oU0"], bcast64(pv["rd"][0:1, :]),
                                        2 * p, nc.gpsimd,
                                    )
                                    norm_mult(
                                        pv["oU1"], bcast64(pv["rd"][1:2, :]),
                                        2 * p + 1, nc.gpsimd,
                                    )
                                    if lastp:
                                        attn_chain_mms(
                                            hp, 0, pv["mine0"], (2,)
                                        )
                                elif kc == 7 and lastp:
                                    # sub1 chains ride a psS-ring tile
                                    # ([P,512] halves = one bank each) so
                                    # psO keeps holding sub0's
                                    spc = psS.tile(
                                        [P, 1024], FP32, tag="sps", name="c1s"
                                    )
                                    pv["c1m"] = {
                                        0: spc[:, 0:512],
                                        1: spc[:, 512:1024],
                                    }
                                    attn_chain_mms(hp, 1, pv["c1m"], (0, 1, 2))
                            if hp == 0 or not lastp:
                                continue
                            # --- last pair tail (short): only the kp3
                            # steps, numerator copies and the two
                            # denominator paths remain ----------------
                            attn_chain_mms(hp, 0, pv["mine0"], (3,))
                            oU0 = copy_out(pv["mine0"])
                            attn_chain_mms(hp, 1, pv["c1m"], (3,))
                            denom_last(oU0, hp, 0)
                            oU1 = copy_out(pv["c1m"])
                            denom_last(oU1, hp, 1)

                    # --- proj + residual + LN2, interleaved per token
                    # tile so LN2 (DVE) overlaps proj (PE) --------------
                    x1t = xt  # x tiles become x1 = x (+ b_proj) + attn
                    with tc.tile_pool(name="gTpool", bufs=1) as gTpool:
                      gT = [
                          gTpool.tile([P, 2, N], FP8, tag=f"gT{j}", name=f"gT{j}")
                          for j in range(HIDP)
                      ]
                      with tc.tile_pool(name="h2Tpool", bufs=1) as h2Tpool:
                        h2T = [
                            h2Tpool.tile([P, 2, N], FP8, tag=f"h2T{j}", name=f"h2T{j}")
                            for j in range(CP)
                        ]
                        with (
                            tc.tile_pool(name="psP", bufs=3, space="PSUM") as psP,
                            tc.tile_pool(name="psT2", bufs=2, space="PSUM") as psT2,
                        ):
                            for m in range(TOK):
                                ps = psP.tile([P, C], FP32, tag="pps", name="pps")
                                for j in range(CP):
                                    for n0, n1 in ((0, 512), (512, 768)):
                                        nc.tensor.matmul(
                                            ps[:, n0:n1],
                                            lhsT=oT[j][:, :, m * P : (m + 1) * P],
                                            rhs=wproj[j][:, :, n0:n1],
                                            start=(j == 0),
                                            stop=(j == CP - 1),
                                            perf_mode=DR,
                                        )
                                nc.vector.scalar_tensor_tensor(
                                    out=xt[m], in0=ps,
                                    scalar=1.0 / (WS_QKV * WS_PROJ),
                                    in1=xt[m],
                                    op0=ALU.mult, op1=ALU.add,
                                )
                                hn = ln_normalize(x1t[m])
                                transpose_affine(hn, h2T, m, g2c, b2c, psT2, "trB")
                                if with_b_fc2:
                                    nc.gpsimd.tensor_tensor(
                                        out=x1t[m], in0=x1t[m], in1=bf2_b, op=ALU.add
                                    )

                        # --- fc1 + gelu -> gT pairs ------------------------
                        # h outer: the h=0 half only needs LN2 of
                        # token tiles 0-3, so it starts ~8us earlier
                        with tc.tile_pool(name="psU", bufs=3, space="PSUM") as psU:
                            # mh pairs share one [128,1024] psum tile so
                            # gelu runs at half the per-call overhead
                            # (bias must be per-partition-constant, so
                            # pairing needs b_fc1 == 0; else fall back);
                            # h-outer keeps the h=0 half starting early
                            # (needs LN2 of token tiles 0-3 only).
                            for h in range(2):
                                for mhp in range(HIDP):
                                    ps = psU.tile([P, 1024], FP32, tag="ups", name="ups")
                                    for i in range(2):
                                        mh = 2 * mhp + i
                                        for j in range(CP):
                                            nc.tensor.matmul(
                                                ps[:, i * 512 : (i + 1) * 512],
                                                lhsT=wfc1[j][:, :, mh * P : (mh + 1) * P],
                                                rhs=h2T[j][:, :, h * 512 : (h + 1) * 512],
                                                start=(j == 0),
                                                stop=(j == CP - 1),
                                                perf_mode=DR,
                                            )
                                    if with_b_fc1:
                                        for i in range(2):
                                            nc.scalar.activation(
                                                out=gT[mhp][:, i, h * 512 : (h + 1) * 512],
                                                in_=ps[:, i * 512 : (i + 1) * 512],
                                                func=AF.Gelu,
                                                bias=bf1c[:, 2 * mhp + i : 2 * mhp + i + 1],
                                                scale=1.0 / WS_FC1,
                                            )
                                    else:
                                        nc.scalar.activation(
                                            out=gT[mhp][:, :, h * 512 : (h + 1) * 512],
                                            in_=ps, func=AF.Gelu,
                                            scale=1.0 / WS_FC1,
                                        )

                      # --- fc2 token-major: out[m] = x1[m] + gT.T @ wfc2 -
                      with tc.tile_pool(name="psY", bufs=2, space="PSUM") as psY:
                        for m in range(TOK):
                            ps = psY.tile([P, C], FP32, tag="yps", name="yps")
                            for j in range(HIDP):
                                for n0, n1 in ((0, 512), (512, 768)):
                                    nc.tensor.matmul(
                                        ps[:, n0:n1],
                                        lhsT=gT[j][:, :, m * P : (m + 1) * P],
                                        rhs=wfc2[j][:, :, n0:n1],
                                        start=(j == 0),
                                        stop=(j == HIDP - 1),
                                        perf_mode=DR,
                                    )
                            nc.vector.scalar_tensor_tensor(
                                out=x1t[m], in0=ps, scalar=1.0 / WS_FC2,
                                in1=x1t[m], op0=ALU.mult, op1=ALU.add,
                            )
                            nc.sync.dma_start(
                                out=out[m * P : (m + 1) * P, :], in_=x1t[m]
                            )

    ctx_lp.__exit__(None, None, None)
    return out


# ---- wait splitting (walrus allows 1 sync wait/instruction) ----

"""Post-pass: this container's walrus rejects >1 sync wait per instruction.

Tile's sem-assignment freely attaches several waits to one instruction.
Peel all but the last wait onto freshly inserted NoOp instructions on the
same engine, placed immediately before the instruction in its block.
"""


def split_multi_waits(nc, max_waits: int = 1) -> int:
    n_split = 0
    for f in nc.m.functions:
        for bb in f.blocks:
            insts = list(bb.instructions)
            out = []
            for inst in insts:
                si = inst.sync_info
                waits = list(si.on_wait) if si is not None else []
                if len(waits) > max_waits:
                    n_split += 1
                    peel = waits[:-max_waits]
                    si.on_wait = waits[-max_waits:]
                    for i in range(0, len(peel), max_waits):
                        nop = mybir.InstNoOp(
                            name=f"I-waitfix-{n_split}-{i}",
                            engine=inst.engine,
                            ins=[],
                            outs=[],
                            sync_info=mybir.SyncInfo(
                                on_wait=peel[i : i + max_waits], on_update=[]
                            ),
                        )
                        nc.register_instruction(nop)
                        out.append(nop)
                out.append(inst)
            if len(out) != len(insts):
                bb.instructions[:] = out
    return n_split


# ----------------------------------------------------------------------
# SPMD entry point: full inputs in, full outputs out (8-way batch-parallel)
# ----------------------------------------------------------------------
import numpy as _np
import ml_dtypes as _mld

_N_CORES = 8
_FP32_KEYS = ["ln1_g", "ln1_b", "b_proj", "ln2_g", "ln2_b", "b_fc1", "b_fc2"]


def _pair_fp8(w):
    """[K, M] fp32 -> [K/2 * 128?, ...] pair layout: out[j*128+p, i*M+c] =
    w[256j+128i+p, c], cast to fp8e4m3."""
    K, M = w.shape
    JP = K // 256
    w8 = w.astype(_mld.float8_e4m3fn)
    w4 = w8.reshape(JP, 2, P, M).transpose(0, 2, 1, 3)  # [j, p, i, c]
    return _np.ascontiguousarray(w4).reshape(JP * P, 2 * M)


def _prep_weights(inputs):
    w = {}
    for k in _FP32_KEYS:
        w[k] = _np.ascontiguousarray(_np.asarray(inputs[k], dtype=_np.float32))
    w["wqkv_p"] = _pair_fp8(_np.asarray(inputs["w_qkv"], dtype=_np.float32) * WS_QKV)
    w["wproj_p"] = _pair_fp8(_np.asarray(inputs["w_proj"], dtype=_np.float32) * WS_PROJ)
    w["wfc1_p"] = _pair_fp8(_np.asarray(inputs["w_fc1"], dtype=_np.float32) * WS_FC1)
    w["wfc2_p"] = _pair_fp8(_np.asarray(inputs["w_fc2"], dtype=_np.float32) * WS_FC2)
    return w


def _build_program(weights):
    import concourse.tile as tile

    nc = bass.Bass("TRN2", target_bir_lowering=False, debug=False,
                   num_devices=_N_CORES)
    with tile.TileContext(nc) as tc:
        build(
            nc, tc,
            with_b_proj=bool(_np.any(weights["b_proj"])),
            with_b_fc2=bool(_np.any(weights["b_fc2"])),
            with_b_fc1=bool(_np.any(weights["b_fc1"])),
        )
    split_multi_waits(nc)
    return nc


def kernel(**inputs):
    from concourse.bass_utils import run_bass_kernel_spmd

    x = _np.ascontiguousarray(_np.asarray(inputs["x"], dtype=_np.float32))
    assert x.shape == (8, N, C), x.shape
    weights = _prep_weights(inputs)
    nc = _build_program(weights)
    in_maps = [{"x": x[b], **weights} for b in range(_N_CORES)]
    res = run_bass_kernel_spmd(nc, in_maps, list(range(_N_CORES)))
    out = _np.stack([res.results[b]["out"] for b in range(_N_CORES)])
    return out.astype(_np.float32)

